# revision 14
# baseline (speedup 1.0000x reference)
"""Trainium2 Bass kernel for Ac4kAttentionOp (int8 q/k + fp8e4m3 v quantized attention).

Shapes: q,k,v [B=2, H=16, N=2048, D=64] fp32 -> out [2,16,2048,64] fp32.
Sharding: 32 (B,H) heads split 4-per-core across 8 NeuronCores; no collectives.

Math (mirrors the reference exactly up to fp32 rounding order):
  k <- k - mean_N(k)
  qq = round(q / sf_q), sf_q = max(amax_D(q)/127, eps)      (per token)
  kq = round(k / sf_k), sf_k = max(amax_D(k)/127, eps)      (per token)
  vq = fp8e4m3(v / sf_v), sf_v = max(amax_N(v)/(448/2.25), eps)  (per channel)
  s^T[m,nq] = sum_d kq[m,d] * (qq[nq,d]*sf_q[nq]*sm) ;  p^T = exp(sf_k[m] * s^T)
  outT[d,nq] = sum_m vq[m,d] * p^T[m,nq] ; denom = ones-column of vq_aug
  out[nq,d] = outT[d,nq] * sf_v[d] / denom[nq]

Performance structure:
  - ACT (exp over all N^2 scores) is the throughput floor (~133us busy/core);
    everything else is arranged to keep it fed back-to-back.
  - All main-loop matmuls (QK and PV) use 128-row fp16 stationaries: kqT/qcsT
    are zero-padded from 64 to 128 contraction rows.  Stationary row-count
    changes between consecutive matmuls serialize LDWEIGHTS (no weight
    preload) and the resulting bubbles pin the PE at its 1.2GHz mid p-state;
    uniform row counts let the PE ramp to 2.4GHz and stay there.
  - Lookahead emission: QK(mt+2)/exp(mt+2) are emitted before PV(mt) so the
    in-order PE queue always has runnable work while ACT computes exp(mt).
  - Per-head prep (quant, DVE) runs during the previous head's half-0 loop;
    prep PE transposes and epilogue transposes are injected as grouped slots
    inside the main loops (ACT's ~2us backlog of queued exps covers the PE
    pause).  Head 0's k/q chains are split into token halves so the first
    QK can issue after roughly half the quant latency.
  - q/v loads and the qcsT parity-split DMAs issue from the Pool (GpSimd)
    DMA queue so transfers overlap the sync-queue ones.
"""
import math
from contextlib import ExitStack

import numpy as np

import concourse.bass as bass
import concourse.tile as tile
from concourse import mybir
from concourse.masks import make_identity

B, H, N, D = 2, 16, 2048, 64
NCORES = 8
HEADS_PER_CORE = (B * H) // NCORES          # 4
SM_SCALE = 1.0 / math.sqrt(D)               # 0.125 (exact power of 2)
MAGIC = 12582912.0                          # 1.5*2^23: fp32 RNE integer round
INT8_MAX = 127.0
F8_AMAX_DIV = float(np.float32(448.0) / np.float32(2.25))  # FP8_MAX / MAX_SCALE
EPS = 1e-8

f32 = mybir.dt.float32
f16 = mybir.dt.float16
f8e4 = mybir.dt.float8e4
ALU = mybir.AluOpType
ACTF = mybir.ActivationFunctionType


def _bc(t: bass.AP, dims, off: int = 0) -> bass.AP:
    """Build a broadcast/restrided view of a tile AP (off in elements)."""
    return bass.AP(tensor=t.tensor, offset=t.offset + off, ap=dims)


def build_attention(nc: bass.Bass, heads: int = HEADS_PER_CORE, n: int = N,
                    bench_loops: int = 0):
    T = n // 128          # token tiles per head
    C = T // 2            # 128-wide transpose chunks
    NQH = n // 2          # query-half width (PSUM budget)
    q_d = nc.dram_tensor("q", [heads, n, D], f32, kind="ExternalInput").ap()
    k_d = nc.dram_tensor("k", [heads, n, D], f32, kind="ExternalInput").ap()
    v_d = nc.dram_tensor("v", [heads, n, D], f32, kind="ExternalInput").ap()
    o_d = nc.dram_tensor("out", [heads, n, D], f32, kind="ExternalOutput").ap()

    with tile.TileContext(nc) as tc, ExitStack() as ctx:
        singles = ctx.enter_context(tc.tile_pool(name="singles", bufs=1))
        loads = ctx.enter_context(tc.tile_pool(name="loads", bufs=2))
        work = ctx.enter_context(tc.tile_pool(name="work", bufs=2))
        scales = ctx.enter_context(tc.tile_pool(name="scales", bufs=2))
        small = ctx.enter_context(tc.tile_pool(name="small", bufs=4))
        opnds = ctx.enter_context(tc.tile_pool(name="opnds", bufs=2))
        pbuf = ctx.enter_context(tc.tile_pool(name="pbuf", bufs=4))
        obuf = ctx.enter_context(tc.tile_pool(name="obuf", bufs=2))
        ostore = ctx.enter_context(tc.tile_pool(name="ostore", bufs=4))
        osb = ctx.enter_context(tc.tile_pool(name="osb", bufs=2))
        ps_s = ctx.enter_context(tc.tile_pool(name="ps_s", bufs=2, space="PSUM"))
        ps_o = ctx.enter_context(tc.tile_pool(name="ps_o", bufs=1, space="PSUM"))
        ps_t = ctx.enter_context(tc.tile_pool(name="ps_t", bufs=2, space="PSUM"))

        ident_f = singles.tile([128, 128], f32)
        make_identity(nc, ident_f)
        ident_h = singles.tile([128, 128], f16)
        make_identity(nc, ident_h)
        ones_row = singles.tile([1, 128], f32)
        nc.gpsimd.memset(ones_row, 1.0)
        # constant [128,128] of 1/n in f16 (2^-11, exact): k-mean matmul weights
        invn_h = singles.tile([128, 128], f16)
        nc.gpsimd.memset(invn_h, 1.0 / n)

        if bench_loops:
            ctx.enter_context(tc.For_i(0, bench_loops, 1))

        # warm the ACT exp table before the first real exp
        warm = singles.tile([1, 1], f32)
        nc.gpsimd.memset(warm, 0.0)
        nc.scalar.activation(warm, warm, ACTF.Exp)

        def load(h):
            """k on the sync DMA queue; q and v on the gpsimd queue so the
            three transfers run in parallel.  k first: it heads the longest
            prep chain (mean -> sub -> quant -> transpose)."""
            k_sb = loads.tile([128, T, D], f32, tag="k_sb")
            nc.sync.dma_start(out=k_sb,
                              in_=k_d[h].rearrange("(t p) d -> p t d", p=128))
            q_sb = loads.tile([128, T, D], f32, tag="q_sb")
            nc.gpsimd.dma_start(out=q_sb,
                                in_=q_d[h].rearrange("(t p) d -> p t d", p=128))
            v_sb = loads.tile([128, T, D], f32, tag="v_sb")
            nc.gpsimd.dma_start(out=v_sb,
                                in_=v_d[h].rearrange("(t p) d -> p t d", p=128))
            return q_sb, k_sb, v_sb

        def prep_cast_k(bufs, tl):
            """DVE: k -> f16 (mean-matmul moving operand)."""
            _, k_sb, _ = bufs
            tl["k_h"] = work.tile([128, T, D], f16, tag="k_h", name="k_h")
            nc.vector.tensor_copy(tl["k_h"], k_sb)

        def mean_pe(tl):
            """PE: column-mean partials via 1/n matmul (same shape family as
            the main-loop matmuls: [128,128] f16 stationary)."""
            mean_ps = ps_s.tile([128, NQH], f32, tag="pss")
            half_td = T * D // 2
            nc.tensor.matmul(mean_ps[:, 0:half_td], invn_h,
                             tl["k_h"][:, 0:T // 2, :], start=True, stop=True)
            nc.tensor.matmul(mean_ps[:, half_td:T * D], invn_h,
                             tl["k_h"][:, T // 2:T, :], start=True, stop=True)
            tl["mean_ps"] = mean_ps

        def prep_mean_red(tl):
            """DVE: reduce mean partials over token tiles."""
            meanb = small.tile([128, D], f32, tag="meanb")
            mean_ps = tl["mean_ps"]
            nc.vector.tensor_reduce(
                out=meanb,
                in_=_bc(mean_ps, [mean_ps.ap[0], [1, D], [D, T]]),
                axis=mybir.AxisListType.X, op=ALU.add)
            tl["meanb"] = meanb

        def quant_int8(x_sb, tagpfx, tl, t0, t1):
            """per-token int8 quantize of tiles [t0,t1); scale/stage tiles in
            tl are allocated on the first part, sub-written on later parts."""
            nt = t1 - t0
            key = tagpfx + "amax"
            if key not in tl:
                tl[key] = scales.tile([128, T], f32, tag=key, name=key)
                tl[tagpfx + "sf"] = scales.tile([128, T], f32,
                                                tag=tagpfx + "sf",
                                                name=tagpfx + "sf")
                tl[tagpfx + "rsf"] = scales.tile([128, T], f32,
                                                 tag=tagpfx + "rsf",
                                                 name=tagpfx + "rsf")
                tl[tagpfx + "xq"] = work.tile([128, T, D], f32,
                                              tag=tagpfx + "xq",
                                              name=tagpfx + "xq")
            amax, sf = tl[key], tl[tagpfx + "sf"]
            rsf, xq = tl[tagpfx + "rsf"], tl[tagpfx + "xq"]
            nc.vector.tensor_reduce(out=amax[:, t0:t1], in_=x_sb[:, t0:t1, :],
                                    axis=mybir.AxisListType.X, op=ALU.max,
                                    apply_absolute_value=True)
            nc.vector.tensor_scalar(out=sf[:, t0:t1], in0=amax[:, t0:t1],
                                    scalar1=1.0 / INT8_MAX, scalar2=EPS,
                                    op0=ALU.mult, op1=ALU.max)
            nc.vector.reciprocal(rsf[:, t0:t1], sf[:, t0:t1])
            nc.vector.tensor_mul(
                xq[:, t0:t1, :], x_sb[:, t0:t1, :],
                _bc(rsf, [rsf.ap[0], [1, nt], [0, D]], off=t0))
            # RNE integer round: (x + MAGIC) - MAGIC
            nc.vector.tensor_scalar(out=xq[:, t0:t1, :], in0=xq[:, t0:t1, :],
                                    scalar1=MAGIC, scalar2=MAGIC,
                                    op0=ALU.add, op1=ALU.subtract)

        def prep_k_chain(bufs, tl, t0=0, t1=None):
            """DVE: mean-sub + int8 quant + f16 cast for k tiles [t0,t1)."""
            _, k_sb, _ = bufs
            t1 = T if t1 is None else t1
            nt = t1 - t0
            if "ks" not in tl:
                tl["ks"] = work.tile([128, T, D], f32, tag="ks", name="ks")
                tl["kq_h"] = work.tile([128, T, D], f16, tag="kq_h", name="kq_h")
            meanb = tl["meanb"]
            nc.vector.tensor_sub(tl["ks"][:, t0:t1, :], k_sb[:, t0:t1, :],
                                 _bc(meanb, [meanb.ap[0], [0, nt], [1, D]]))
            quant_int8(tl["ks"], "k", tl, t0, t1)
            nc.vector.tensor_copy(tl["kq_h"][:, t0:t1, :],
                                  tl["kxq"][:, t0:t1, :])

        def prep_q_chain(bufs, tl, t0=0, t1=None):
            """DVE: int8 quant + fold sf_q*sm + f16 cast for q tiles."""
            q_sb, _, _ = bufs
            t1 = T if t1 is None else t1
            nt = t1 - t0
            if "qcs_h" not in tl:
                tl["csfq"] = scales.tile([128, T], f32, tag="csfq", name="csfq")
                tl["qcs"] = work.tile([128, T, D], f32, tag="qcs", name="qcs")
                tl["qcs_h"] = work.tile([128, T, D], f16, tag="qcs_h", name="qcs_h")
            quant_int8(q_sb, "q", tl, t0, t1)
            csfq = tl["csfq"]
            nc.vector.tensor_scalar_mul(csfq[:, t0:t1], tl["qsf"][:, t0:t1],
                                        SM_SCALE)
            nc.vector.tensor_mul(
                tl["qcs"][:, t0:t1, :], tl["qxq"][:, t0:t1, :],
                _bc(csfq, [csfq.ap[0], [1, nt], [0, D]], off=t0))
            nc.vector.tensor_copy(tl["qcs_h"][:, t0:t1, :],
                                  tl["qcs"][:, t0:t1, :])

        def prep_v_amax(bufs, tl):
            """DVE: per-channel |v| max partials (channel-major view)."""
            _, _, v_sb = bufs
            amax_vp = work.tile([128, D], f32, tag="amax_vp")
            nc.vector.tensor_reduce(
                out=amax_vp,
                in_=_bc(v_sb, [v_sb.ap[0], [1, D], [D, T]]),
                axis=mybir.AxisListType.X, op=ALU.max,
                apply_absolute_value=True)
            tl["amax_vp"] = amax_vp

        def transpose_group(src_key, dst_key, tag, queue, tl, c0=0, c1=None):
            """PE chunk transposes (parity-stacked via DVE) of chunks [c0,c1)
            then two strided parity-split DMAs into the top half of the
            zero-padded [128,(T,128)] operand."""
            c1 = C if c1 is None else c1
            if dst_key not in tl:
                tl[dst_key] = opnds.tile([128, T, 128], f16, tag=tag, name=tag)
                nc.gpsimd.memset(tl[dst_key][64:128, :, :], 0.0)
                tl[dst_key + "_st"] = work.tile([128, C, 128], f16,
                                                tag=tag + "_st",
                                                name=tag + "_st")
            dstT, stk = tl[dst_key], tl[dst_key + "_st"]
            x_h = tl[src_key]
            for c in range(c0, c1):
                tp = ps_t.tile([128, 128], f16, tag="pst")
                nc.tensor.transpose(tp, x_h[:, 2 * c:2 * c + 2, :], ident_h)
                nc.vector.tensor_copy(stk[:, c, :], tp)
            eng = nc.sync if queue == "sync" else nc.gpsimd
            d64 = dstT[0:64]
            nci = c1 - c0
            eng.dma_start(
                out=_bc(d64, [d64.ap[0], [2 * 128, nci], [1, 128]],
                        off=c0 * 256),
                in_=stk[0:64, c0:c1, :])
            eng.dma_start(
                out=_bc(d64, [d64.ap[0], [2 * 128, nci], [1, 128]],
                        off=c0 * 256 + 128),
                in_=stk[64:128, c0:c1, :])

        def prep_v_scale_pre(tl):
            """PE transpose of amax partials + DVE scale math (all small)."""
            vt_ps = ps_t.tile([D, 128], f32, tag="pst")
            nc.tensor.transpose(vt_ps, tl["amax_vp"], ident_f)
            amax_vT = scales.tile([D, 1], f32, tag="amax_vT")
            nc.vector.tensor_reduce(out=amax_vT, in_=vt_ps,
                                    axis=mybir.AxisListType.X, op=ALU.max)
            sf_vT = scales.tile([D, 1], f32, tag="sf_vT")
            nc.vector.tensor_scalar(out=sf_vT, in0=amax_vT,
                                    scalar1=1.0 / F8_AMAX_DIV, scalar2=EPS,
                                    op0=ALU.mult, op1=ALU.max)
            rsf_vT = scales.tile([D, 1], f32, tag="rsf_vT")
            nc.vector.reciprocal(rsf_vT, sf_vT)
            sfv65 = scales.tile([65, 1], f32, tag="sfv65")
            nc.gpsimd.memset(sfv65, 1.0)
            nc.vector.tensor_copy(sfv65[0:D, :], sf_vT)
            rsf_row = small.tile([1, D], f32, tag="rsf_row")
            nc.sync.dma_start(out=rsf_row, in_=rsf_vT)
            rsf_bps = ps_t.tile([128, D], f32, tag="pst")
            nc.tensor.matmul(rsf_bps, ones_row, rsf_row, start=True, stop=True)
            rsf_b = small.tile([128, D], f32, tag="rsf_b")
            nc.vector.tensor_copy(rsf_b, rsf_bps)
            tl["rsf_b"] = rsf_b
            tl["sfv65"] = sfv65

        def prep_v_quant(bufs, tl, pool=True):
            """fp8 quantize v + build augmented (ones-column) operand.  The
            two big elementwise ops go to Pool in steady state (keeps DVE
            free so the o_ps scale fires promptly); the f8->f16 copy stays
            on DVE."""
            _, _, v_sb = bufs
            rsf_b = tl["rsf_b"]
            eng = nc.gpsimd if pool else nc.vector
            vq_pre = work.tile([128, T, D], f32, tag="vq_pre")
            eng.tensor_mul(vq_pre, v_sb,
                           _bc(rsf_b, [rsf_b.ap[0], [0, T], [1, D]]))
            vq_f8 = work.tile([128, T, D], f8e4, tag="vq_f8")
            eng.tensor_copy(vq_f8, vq_pre)
            vq_aug = opnds.tile([128, T, D + 1], f16, tag="vq_aug")
            nc.vector.tensor_copy(vq_aug[:, :, 0:D], vq_f8)
            nc.gpsimd.memset(vq_aug[:, :, D:D + 1], 1.0)
            tl["vq_aug"] = vq_aug

        def half_loop(h, tl, half, slots=None):
            """Main QK->exp->PV loop for one query half (NQH queries).
            Lookahead: QK(mt+2)/exp(mt+2) emitted before PV(mt).
            slots: {mt: [closure,...]} run after qk_exp(mt+2) is emitted."""
            slots = slots or {}
            kqT, qcsT, vq_aug = tl["kqT"], tl["qcsT"], tl["vq_aug"]
            sf_k = tl["ksf"]
            TH = T // 2

            def qk_exp(mt):
                s_ps = ps_s.tile([128, NQH], f32, tag="pss")
                for j in range(NQH // 512):
                    rhs = qcsT[:, half * TH + 4 * j:half * TH + 4 * (j + 1), :]
                    nc.tensor.matmul(s_ps[:, j * 512:(j + 1) * 512],
                                     kqT[:, mt, :], rhs, start=True, stop=True)
                p_sb = pbuf.tile([128, NQH], f16, tag="p_sb")
                nc.scalar.activation(p_sb, s_ps, ACTF.Exp,
                                     scale=sf_k[:, mt:mt + 1])
                return p_sb

            o_ps = ps_o.tile([65, NQH], f32, tag="pso")
            ps = [qk_exp(0), qk_exp(1)]
            for mt in range(T):
                if mt + 2 < T:
                    ps.append(qk_exp(mt + 2))
                for fn in slots.get(mt, ()):
                    fn()
                p_sb = ps[mt]
                for j in range(NQH // 512):
                    nc.tensor.matmul(
                        o_ps[:, j * 512:(j + 1) * 512],
                        vq_aug[:, mt, :],
                        p_sb[:, j * 512:(j + 1) * 512],
                        start=(mt == 0), stop=(mt == T - 1))
            # scale by per-channel v scale, park in SBUF (frees the psum bank)
            outT_sb = obuf.tile([65, NQH], f32, tag="outT")
            nc.vector.tensor_scalar_mul(outT_sb, o_ps, tl["sfv65"][:, 0:1])
            return outT_sb

        def epilogue_half(outT_sb, out_sb, half):
            """Out-transposes + denominator divide for one query half."""
            def chunks():
                for c in range(NQH // 128):
                    tp2 = ps_t.tile([128, 65], f32, tag="pst")
                    nc.tensor.transpose(tp2, outT_sb[:, c * 128:(c + 1) * 128],
                                        ident_f[0:65, 0:65])
                    rec = ostore.tile([128, 1], f32, tag="rec")
                    nc.vector.reciprocal(rec, tp2[:, D:D + 1])
                    nc.vector.tensor_mul(
                        out_sb[:, half * (T // 2) + c, :], tp2[:, 0:D],
                        _bc(rec, [rec.ap[0], [0, D]]))
            return chunks

        # ---- head pipeline ----
        # Steady-state emission for head h:
        #   half0(h): slot2 = mean matmuls (h+1, PE) + epilogue chunks of
        #             h-1 half1 + store(h-1); DVE runs the k+q+v quant
        #             chains of h+1 underneath (emitted in slots).
        #   half1(h): slot1/5 = kq/qcs transpose groups (h+1);
        #             slot3 = epilogue chunks of h half0; slot9 = v-scale.
        # Head 0: k/q chains split into token halves so QK(0) issues early.
        tl = {}
        bufs = load(0)
        prep_cast_k(bufs, tl)
        mean_pe(tl)
        prep_mean_red(tl)
        TH2 = T // 2
        prep_k_chain(bufs, tl, 0, TH2)
        transpose_group("kq_h", "kqT", "kqT", "sync", tl, 0, C // 2)
        prep_q_chain(bufs, tl, 0, TH2)
        transpose_group("qcs_h", "qcsT", "qcsT", "gpsimd", tl, 0, C // 2)
        prep_v_amax(bufs, tl)
        prep_v_scale_pre(tl)
        prep_v_quant(bufs, tl, pool=False)

        def k_part2(bufs=bufs, tl=tl):
            prep_k_chain(bufs, tl, TH2, T)

        def kT_part2(tl=tl):
            transpose_group("kq_h", "kqT", "kqT", "sync", tl, C // 2, C)

        def q_part2(bufs=bufs, tl=tl):
            prep_q_chain(bufs, tl, TH2, T)

        def qT_part2(tl=tl):
            transpose_group("qcs_h", "qcsT", "qcsT", "gpsimd", tl, C // 2, C)

        h0_slots0 = {0: [k_part2], 2: [kT_part2], 4: [q_part2],
                     6: [qT_part2]}

        prev_chunks1 = None    # epilogue closure: half1 of previous head
        prev_store = None
        for h in range(heads):
            has_next = h + 1 < heads
            out_sb = osb.tile([128, T, D], f32, tag="out_sb")
            slots0 = dict(h0_slots0) if h == 0 else {}
            h0_slots0 = {}
            if prev_chunks1 is not None:
                slots0.setdefault(3, []).append(prev_chunks1)
                slots0.setdefault(3, []).append(prev_store)
            tl_n = {}
            if has_next:
                bufs_n = load(h + 1)

                def s_cast(bufs_n=bufs_n, tl_n=tl_n):
                    prep_cast_k(bufs_n, tl_n)

                def s_mean(tl_n=tl_n):
                    mean_pe(tl_n)
                    prep_mean_red(tl_n)

                def s_kchain(bufs_n=bufs_n, tl_n=tl_n):
                    prep_k_chain(bufs_n, tl_n)

                def s_qchain(bufs_n=bufs_n, tl_n=tl_n):
                    prep_q_chain(bufs_n, tl_n)

                def s_vamax(bufs_n=bufs_n, tl_n=tl_n):
                    prep_v_amax(bufs_n, tl_n)

                slots0.setdefault(1, []).append(s_cast)
                slots0.setdefault(2, []).append(s_mean)
                slots0.setdefault(4, []).append(s_kchain)
                slots0.setdefault(7, []).append(s_qchain)
                slots0.setdefault(10, []).append(s_vamax)
            outT0 = half_loop(h, tl, 0, slots0)

            def store_half(h=h, out_sb=out_sb):
                def _st(half):
                    def go():
                        nc.sync.dma_start(
                            out=o_d[h, half * NQH:(half + 1) * NQH, :]
                            .rearrange("(t p) d -> p t d", p=128),
                            in_=out_sb[:, half * (T // 2):(half + 1) * (T // 2), :])
                    return go
                return _st
            st_mk = store_half()
            slots1 = {}
            slots1[3] = [epilogue_half(outT0, out_sb, 0)]
            slots1[8] = [st_mk(0)]
            if has_next:
                def s_kqT(tl_n=tl_n):
                    transpose_group("kq_h", "kqT", "kqT", "sync", tl_n)

                def s_qcsT(tl_n=tl_n):
                    transpose_group("qcs_h", "qcsT", "qcsT", "gpsimd", tl_n)

                def s_vpre(tl_n=tl_n):
                    prep_v_scale_pre(tl_n)

                def s_vquant(bufs_n=bufs_n, tl_n=tl_n):
                    prep_v_quant(bufs_n, tl_n, pool=True)

                slots1[1] = [s_kqT]
                slots1[2] = [s_vpre]
                slots1[4] = [s_vquant]
                slots1[6] = [s_qcsT]
            outT1 = half_loop(h, tl, 1, slots1)
            prev_chunks1 = epilogue_half(outT1, out_sb, 1)
            prev_store = st_mk(1)
            if has_next:
                tl = tl_n
        # last head's half-1 epilogue tail
        prev_chunks1()
        prev_store()
    return nc


_CACHED = {}


def _get_nc():
    if "nc" not in _CACHED:
        from concourse import bacc

        nc = bacc.Bacc("TRN2", target_bir_lowering=False, debug=False)
        build_attention(nc)
        nc.compile()
        _CACHED["nc"] = nc
    return _CACHED["nc"]


def kernel(q: np.ndarray, k: np.ndarray, v: np.ndarray) -> np.ndarray:
    from concourse.bass_utils import run_bass_kernel_spmd

    nc = _get_nc()
    qf = np.ascontiguousarray(np.asarray(q, dtype=np.float32).reshape(B * H, N, D))
    kf = np.ascontiguousarray(np.asarray(k, dtype=np.float32).reshape(B * H, N, D))
    vf = np.ascontiguousarray(np.asarray(v, dtype=np.float32).reshape(B * H, N, D))
    hpc = HEADS_PER_CORE
    in_maps = [
        {"q": qf[c * hpc:(c + 1) * hpc],
         "k": kf[c * hpc:(c + 1) * hpc],
         "v": vf[c * hpc:(c + 1) * hpc]}
        for c in range(NCORES)
    ]
    res = run_bass_kernel_spmd(nc, in_maps, core_ids=list(range(NCORES)))
    out = np.concatenate([np.asarray(r["out"]) for r in res.results], axis=0)
    return out.reshape(B, H, N, D).astype(np.float32)


# revision 15
# speedup vs baseline: 1.2014x; 1.2014x over previous
"""Trainium2 Bass kernel for Ac4kAttentionOp (int8 q/k + fp8e4m3 v quantized attention).

Shapes: q,k,v [B=2, H=16, N=2048, D=64] fp32 -> out [2,16,2048,64] fp32.
Sharding: 32 (B,H) heads split 4-per-core across 8 NeuronCores; no collectives.

Math (mirrors the reference exactly up to fp32 rounding order):
  k <- k - mean_N(k)
  qq = round(q / sf_q), sf_q = max(amax_D(q)/127, eps)      (per token)
  kq = round(k / sf_k), sf_k = max(amax_D(k)/127, eps)      (per token)
  vq = fp8e4m3(v / sf_v), sf_v = max(amax_N(v)/(448/2.25), eps)  (per channel)
  s^T[m,nq] = sum_d kq[m,d] * (qq[nq,d]*sf_q[nq]*sm) ;  p^T = exp(sf_k[m] * s^T)
  outT[d,nq] = sum_m vq[m,d] * p^T[m,nq] ; denom = ones-column of vq_aug
  out[nq,d] = outT[d,nq] * sf_v[d] / denom[nq]

Performance structure:
  - ACT (exp over all N^2 scores) is the throughput floor (~133us busy/core);
    everything else is arranged to keep it fed back-to-back.
  - All main-loop matmuls (QK and PV) use 128-row fp16 stationaries: kqT/qcsT
    are zero-padded from 64 to 128 contraction rows.  Stationary row-count
    changes between consecutive matmuls serialize LDWEIGHTS (no weight
    preload) and the resulting bubbles pin the PE at its 1.2GHz mid p-state;
    uniform row counts let the PE ramp to 2.4GHz and stay there.
  - Lookahead emission: QK(mt+2)/exp(mt+2) are emitted before PV(mt) so the
    in-order PE queue always has runnable work while ACT computes exp(mt).
  - Per-head prep (quant, DVE) runs during the previous head's half-0 loop;
    prep PE transposes and epilogue transposes are injected as grouped slots
    inside the main loops (ACT's ~2us backlog of queued exps covers the PE
    pause).  Head 0's k/q chains are split into token halves so the first
    QK can issue after roughly half the quant latency.
  - q/v loads and the qcsT parity-split DMAs issue from the Pool (GpSimd)
    DMA queue so transfers overlap the sync-queue ones.
"""
import math
from contextlib import ExitStack

import numpy as np

import concourse.bass as bass
import concourse.tile as tile
from concourse import mybir
from concourse.masks import make_identity

B, H, N, D = 2, 16, 2048, 64
NCORES = 8
HEADS_PER_CORE = (B * H) // NCORES          # 4
SM_SCALE = 1.0 / math.sqrt(D)               # 0.125 (exact power of 2)
MAGIC = 12582912.0                          # 1.5*2^23: fp32 RNE integer round
INT8_MAX = 127.0
F8_AMAX_DIV = float(np.float32(448.0) / np.float32(2.25))  # FP8_MAX / MAX_SCALE
EPS = 1e-8

f32 = mybir.dt.float32
f16 = mybir.dt.float16
f8e4 = mybir.dt.float8e4
ALU = mybir.AluOpType
ACTF = mybir.ActivationFunctionType


def _bc(t: bass.AP, dims, off: int = 0) -> bass.AP:
    """Build a broadcast/restrided view of a tile AP (off in elements)."""
    return bass.AP(tensor=t.tensor, offset=t.offset + off, ap=dims)


def build_attention(nc: bass.Bass, heads: int = HEADS_PER_CORE, n: int = N,
                    bench_loops: int = 0):
    T = n // 128          # token tiles per head
    C = T // 2            # 128-wide transpose chunks
    NQH = n // 2          # query-half width (PSUM budget)
    q_d = nc.dram_tensor("q", [heads, n, D], f32, kind="ExternalInput").ap()
    k_d = nc.dram_tensor("k", [heads, n, D], f32, kind="ExternalInput").ap()
    v_d = nc.dram_tensor("v", [heads, n, D], f32, kind="ExternalInput").ap()
    o_d = nc.dram_tensor("out", [heads, n, D], f32, kind="ExternalOutput").ap()

    with tile.TileContext(nc) as tc, ExitStack() as ctx:
        singles = ctx.enter_context(tc.tile_pool(name="singles", bufs=1))
        loads = ctx.enter_context(tc.tile_pool(name="loads", bufs=2))
        work = ctx.enter_context(tc.tile_pool(name="work", bufs=2))
        scales = ctx.enter_context(tc.tile_pool(name="scales", bufs=2))
        small = ctx.enter_context(tc.tile_pool(name="small", bufs=4))
        opnds = ctx.enter_context(tc.tile_pool(name="opnds", bufs=2))
        pbuf = ctx.enter_context(tc.tile_pool(name="pbuf", bufs=4))
        obuf = ctx.enter_context(tc.tile_pool(name="obuf", bufs=2))
        ostore = ctx.enter_context(tc.tile_pool(name="ostore", bufs=4))
        osb = ctx.enter_context(tc.tile_pool(name="osb", bufs=2))
        ps_s = ctx.enter_context(tc.tile_pool(name="ps_s", bufs=2, space="PSUM"))
        ps_o = ctx.enter_context(tc.tile_pool(name="ps_o", bufs=1, space="PSUM"))
        ps_t = ctx.enter_context(tc.tile_pool(name="ps_t", bufs=2, space="PSUM"))

        ident_f = singles.tile([128, 128], f32)
        make_identity(nc, ident_f)
        ident_h = singles.tile([128, 128], f16)
        make_identity(nc, ident_h)
        ones_row = singles.tile([1, 128], f32)
        nc.gpsimd.memset(ones_row, 1.0)
        # constant [128,128] of 1/n in f16 (2^-11, exact): k-mean matmul weights
        invn_h = singles.tile([128, 128], f16)
        nc.gpsimd.memset(invn_h, 1.0 / n)

        if bench_loops:
            ctx.enter_context(tc.For_i(0, bench_loops, 1))

        # warm the ACT exp table before the first real exp
        warm = singles.tile([1, 1], f32)
        nc.gpsimd.memset(warm, 0.0)
        nc.scalar.activation(warm, warm, ACTF.Exp)

        def load(h):
            """k on the sync DMA queue; q and v on the gpsimd queue so the
            three transfers run in parallel.  k first: it heads the longest
            prep chain (mean -> sub -> quant -> transpose)."""
            k_sb = loads.tile([128, T, D], f32, tag="k_sb")
            nc.sync.dma_start(out=k_sb,
                              in_=k_d[h].rearrange("(t p) d -> p t d", p=128))
            q_sb = loads.tile([128, T, D], f32, tag="q_sb")
            nc.gpsimd.dma_start(out=q_sb,
                                in_=q_d[h].rearrange("(t p) d -> p t d", p=128))
            v_sb = loads.tile([128, T, D], f32, tag="v_sb")
            nc.gpsimd.dma_start(out=v_sb,
                                in_=v_d[h].rearrange("(t p) d -> p t d", p=128))
            return q_sb, k_sb, v_sb

        def prep_cast_k(bufs, tl):
            """DVE: k -> f16 (mean-matmul moving operand)."""
            _, k_sb, _ = bufs
            tl["k_h"] = work.tile([128, T, D], f16, tag="k_h", name="k_h")
            nc.vector.tensor_copy(tl["k_h"], k_sb)

        def mean_pe(tl):
            """PE: column-mean partials via 1/n matmul (same shape family as
            the main-loop matmuls: [128,128] f16 stationary)."""
            mean_ps = ps_s.tile([128, NQH], f32, tag="pss")
            half_td = T * D // 2
            nc.tensor.matmul(mean_ps[:, 0:half_td], invn_h,
                             tl["k_h"][:, 0:T // 2, :], start=True, stop=True)
            nc.tensor.matmul(mean_ps[:, half_td:T * D], invn_h,
                             tl["k_h"][:, T // 2:T, :], start=True, stop=True)
            tl["mean_ps"] = mean_ps

        def prep_mean_red(tl):
            """DVE: reduce mean partials over token tiles."""
            meanb = small.tile([128, D], f32, tag="meanb")
            mean_ps = tl["mean_ps"]
            nc.vector.tensor_reduce(
                out=meanb,
                in_=_bc(mean_ps, [mean_ps.ap[0], [1, D], [D, T]]),
                axis=mybir.AxisListType.X, op=ALU.add)
            tl["meanb"] = meanb

        def quant_int8(x_sb, tagpfx, tl, t0, t1):
            """per-token int8 quantize of tiles [t0,t1); scale/stage tiles in
            tl are allocated on the first part, sub-written on later parts."""
            nt = t1 - t0
            key = tagpfx + "amax"
            if key not in tl:
                tl[key] = scales.tile([128, T], f32, tag=key, name=key)
                tl[tagpfx + "sf"] = scales.tile([128, T], f32,
                                                tag=tagpfx + "sf",
                                                name=tagpfx + "sf")
                tl[tagpfx + "rsf"] = scales.tile([128, T], f32,
                                                 tag=tagpfx + "rsf",
                                                 name=tagpfx + "rsf")
                tl[tagpfx + "xq"] = work.tile([128, T, D], f32,
                                              tag=tagpfx + "xq",
                                              name=tagpfx + "xq")
            amax, sf = tl[key], tl[tagpfx + "sf"]
            rsf, xq = tl[tagpfx + "rsf"], tl[tagpfx + "xq"]
            nc.vector.tensor_reduce(out=amax[:, t0:t1], in_=x_sb[:, t0:t1, :],
                                    axis=mybir.AxisListType.X, op=ALU.max,
                                    apply_absolute_value=True)
            nc.vector.tensor_scalar(out=sf[:, t0:t1], in0=amax[:, t0:t1],
                                    scalar1=1.0 / INT8_MAX, scalar2=EPS,
                                    op0=ALU.mult, op1=ALU.max)
            nc.vector.reciprocal(rsf[:, t0:t1], sf[:, t0:t1])
            nc.vector.tensor_mul(
                xq[:, t0:t1, :], x_sb[:, t0:t1, :],
                _bc(rsf, [rsf.ap[0], [1, nt], [0, D]], off=t0))
            # RNE integer round: (x + MAGIC) - MAGIC
            nc.vector.tensor_scalar(out=xq[:, t0:t1, :], in0=xq[:, t0:t1, :],
                                    scalar1=MAGIC, scalar2=MAGIC,
                                    op0=ALU.add, op1=ALU.subtract)

        def prep_k_chain(bufs, tl, t0=0, t1=None):
            """DVE: mean-sub + int8 quant + f16 cast for k tiles [t0,t1)."""
            _, k_sb, _ = bufs
            t1 = T if t1 is None else t1
            nt = t1 - t0
            if "ks" not in tl:
                tl["ks"] = work.tile([128, T, D], f32, tag="ks", name="ks")
                tl["kq_h"] = work.tile([128, T, D], f16, tag="kq_h", name="kq_h")
            meanb = tl["meanb"]
            nc.vector.tensor_sub(tl["ks"][:, t0:t1, :], k_sb[:, t0:t1, :],
                                 _bc(meanb, [meanb.ap[0], [0, nt], [1, D]]))
            quant_int8(tl["ks"], "k", tl, t0, t1)
            nc.vector.tensor_copy(tl["kq_h"][:, t0:t1, :],
                                  tl["kxq"][:, t0:t1, :])

        def prep_q_chain(bufs, tl, t0=0, t1=None):
            """DVE: int8 quant + fold sf_q*sm + f16 cast for q tiles."""
            q_sb, _, _ = bufs
            t1 = T if t1 is None else t1
            nt = t1 - t0
            if "qcs_h" not in tl:
                tl["csfq"] = scales.tile([128, T], f32, tag="csfq", name="csfq")
                tl["qcs"] = work.tile([128, T, D], f32, tag="qcs", name="qcs")
                tl["qcs_h"] = work.tile([128, T, D], f16, tag="qcs_h", name="qcs_h")
            quant_int8(q_sb, "q", tl, t0, t1)
            csfq = tl["csfq"]
            nc.vector.tensor_scalar_mul(csfq[:, t0:t1], tl["qsf"][:, t0:t1],
                                        SM_SCALE)
            nc.vector.tensor_mul(
                tl["qcs"][:, t0:t1, :], tl["qxq"][:, t0:t1, :],
                _bc(csfq, [csfq.ap[0], [1, nt], [0, D]], off=t0))
            nc.vector.tensor_copy(tl["qcs_h"][:, t0:t1, :],
                                  tl["qcs"][:, t0:t1, :])

        def prep_v_amax(bufs, tl):
            """DVE: per-channel |v| max partials (channel-major view)."""
            _, _, v_sb = bufs
            amax_vp = work.tile([128, D], f32, tag="amax_vp")
            nc.vector.tensor_reduce(
                out=amax_vp,
                in_=_bc(v_sb, [v_sb.ap[0], [1, D], [D, T]]),
                axis=mybir.AxisListType.X, op=ALU.max,
                apply_absolute_value=True)
            tl["amax_vp"] = amax_vp

        def alloc_padded(dst_key, tag, tl):
            """Allocate a zero-padded transposed operand + its stack buffer;
            the Pool memset of the pad runs early, off the critical path."""
            tl[dst_key] = opnds.tile([128, T, 128], f16, tag=tag, name=tag)
            nc.gpsimd.memset(tl[dst_key][64:128, :, :], 0.0)
            tl[dst_key + "_st"] = work.tile([128, C, 128], f16,
                                            tag=tag + "_st",
                                            name=tag + "_st")

        def transpose_group(src_key, dst_key, tag, queue, tl, c0=0, c1=None):
            """PE chunk transposes (parity-stacked via DVE) of chunks [c0,c1)
            then two strided parity-split DMAs into the top half of the
            zero-padded [128,(T,128)] operand."""
            c1 = C if c1 is None else c1
            if dst_key not in tl:
                alloc_padded(dst_key, tag, tl)
            dstT, stk = tl[dst_key], tl[dst_key + "_st"]
            x_h = tl[src_key]
            for c in range(c0, c1):
                tp = ps_t.tile([128, 128], f16, tag="pst")
                nc.tensor.transpose(tp, x_h[:, 2 * c:2 * c + 2, :], ident_h)
                nc.vector.tensor_copy(stk[:, c, :], tp)
            eng = nc.sync if queue == "sync" else nc.gpsimd
            d64 = dstT[0:64]
            nci = c1 - c0
            eng.dma_start(
                out=_bc(d64, [d64.ap[0], [2 * 128, nci], [1, 128]],
                        off=c0 * 256),
                in_=stk[0:64, c0:c1, :])
            eng.dma_start(
                out=_bc(d64, [d64.ap[0], [2 * 128, nci], [1, 128]],
                        off=c0 * 256 + 128),
                in_=stk[64:128, c0:c1, :])

        def prep_v_scale_pre(tl):
            """PE transpose of amax partials + DVE scale math (all small)."""
            vt_ps = ps_t.tile([D, 128], f32, tag="pst")
            nc.tensor.transpose(vt_ps, tl["amax_vp"], ident_f)
            amax_vT = scales.tile([D, 1], f32, tag="amax_vT")
            nc.vector.tensor_reduce(out=amax_vT, in_=vt_ps,
                                    axis=mybir.AxisListType.X, op=ALU.max)
            sf_vT = scales.tile([D, 1], f32, tag="sf_vT")
            nc.vector.tensor_scalar(out=sf_vT, in0=amax_vT,
                                    scalar1=1.0 / F8_AMAX_DIV, scalar2=EPS,
                                    op0=ALU.mult, op1=ALU.max)
            rsf_vT = scales.tile([D, 1], f32, tag="rsf_vT")
            nc.vector.reciprocal(rsf_vT, sf_vT)
            sfv65 = scales.tile([65, 1], f32, tag="sfv65")
            nc.gpsimd.memset(sfv65, 1.0)
            nc.vector.tensor_copy(sfv65[0:D, :], sf_vT)
            rsf_row = small.tile([1, D], f32, tag="rsf_row")
            nc.sync.dma_start(out=rsf_row, in_=rsf_vT)
            rsf_bps = ps_t.tile([128, D], f32, tag="pst")
            nc.tensor.matmul(rsf_bps, ones_row, rsf_row, start=True, stop=True)
            rsf_b = small.tile([128, D], f32, tag="rsf_b")
            nc.vector.tensor_copy(rsf_b, rsf_bps)
            tl["rsf_b"] = rsf_b
            tl["sfv65"] = sfv65

        def prep_v_quant(bufs, tl, pool=True):
            """fp8 quantize v (scale-multiply + cast to the f8e4 grid).  The
            two big elementwise ops go to Pool in steady state (keeps DVE
            free so the o_ps scale fires promptly)."""
            _, _, v_sb = bufs
            rsf_b = tl["rsf_b"]
            eng = nc.gpsimd if pool else nc.vector
            vq_pre = work.tile([128, T, D], f32, tag="vq_pre")
            eng.tensor_mul(vq_pre, v_sb,
                           _bc(rsf_b, [rsf_b.ap[0], [0, T], [1, D]]))
            vq_f8 = work.tile([128, T, D], f8e4, tag="vq_f8")
            eng.tensor_copy(vq_f8, vq_pre)
            tl["vq_f8"] = vq_f8

        def prep_v_aug(tl):
            """DVE: f8 grid values -> f16 PV stationary + ones column."""
            vq_aug = opnds.tile([128, T, D + 1], f16, tag="vq_aug")
            nc.vector.tensor_copy(vq_aug[:, :, 0:D], tl["vq_f8"])
            nc.gpsimd.memset(vq_aug[:, :, D:D + 1], 1.0)
            tl["vq_aug"] = vq_aug

        def half_loop(h, tl, half, slots=None):
            """Main QK->exp->PV loop for one query half (NQH queries).
            Lookahead: QK(mt+2)/exp(mt+2) emitted before PV(mt).
            slots: {mt: [closure,...]} run after qk_exp(mt+2) is emitted."""
            slots = slots or {}
            kqT, qcsT, vq_aug = tl["kqT"], tl["qcsT"], tl["vq_aug"]
            sf_k = tl["ksf"]
            TH = T // 2

            def qk_exp(mt):
                s_ps = ps_s.tile([128, NQH], f32, tag="pss")
                for j in range(NQH // 512):
                    rhs = qcsT[:, half * TH + 4 * j:half * TH + 4 * (j + 1), :]
                    nc.tensor.matmul(s_ps[:, j * 512:(j + 1) * 512],
                                     kqT[:, mt, :], rhs, start=True, stop=True)
                p_sb = pbuf.tile([128, NQH], f16, tag="p_sb")
                nc.scalar.activation(p_sb, s_ps, ACTF.Exp,
                                     scale=sf_k[:, mt:mt + 1])
                return p_sb

            o_ps = ps_o.tile([65, NQH], f32, tag="pso")
            ps = [qk_exp(0), qk_exp(1)]
            for mt in range(T):
                if mt + 2 < T:
                    ps.append(qk_exp(mt + 2))
                for fn in slots.get(mt, ()):
                    fn()
                p_sb = ps[mt]
                for j in range(NQH // 512):
                    nc.tensor.matmul(
                        o_ps[:, j * 512:(j + 1) * 512],
                        vq_aug[:, mt, :],
                        p_sb[:, j * 512:(j + 1) * 512],
                        start=(mt == 0), stop=(mt == T - 1))
            # scale by per-channel v scale, park in SBUF (frees the psum bank)
            outT_sb = obuf.tile([65, NQH], f32, tag="outT")
            nc.vector.tensor_scalar_mul(outT_sb, o_ps, tl["sfv65"][:, 0:1])
            return outT_sb

        def epilogue_half(outT_sb, out_sb, half):
            """Out-transposes + denominator divide for one query half."""
            def chunks():
                for c in range(NQH // 128):
                    tp2 = ps_t.tile([128, 65], f32, tag="pst")
                    nc.tensor.transpose(tp2, outT_sb[:, c * 128:(c + 1) * 128],
                                        ident_f[0:65, 0:65])
                    rec = ostore.tile([128, 1], f32, tag="rec")
                    nc.vector.reciprocal(rec, tp2[:, D:D + 1])
                    nc.vector.tensor_mul(
                        out_sb[:, half * (T // 2) + c, :], tp2[:, 0:D],
                        _bc(rec, [rec.ap[0], [0, D]]))
            return chunks

        # ---- head pipeline ----
        # Steady-state emission for head h:
        #   half0(h): slot2 = mean matmuls (h+1, PE) + epilogue chunks of
        #             h-1 half1 + store(h-1); DVE runs the k+q+v quant
        #             chains of h+1 underneath (emitted in slots).
        #   half1(h): slot1/5 = kq/qcs transpose groups (h+1);
        #             slot3 = epilogue chunks of h half0; slot9 = v-scale.
        # Head 0: k/q chains split into token halves so QK(0) issues early.
        tl = {}
        bufs = load(0)
        prep_cast_k(bufs, tl)
        mean_pe(tl)
        prep_mean_red(tl)
        TH2 = T // 2
        prep_k_chain(bufs, tl, 0, TH2)
        transpose_group("kq_h", "kqT", "kqT", "sync", tl, 0, C // 2)
        prep_q_chain(bufs, tl, 0, TH2)
        transpose_group("qcs_h", "qcsT", "qcsT", "gpsimd", tl, 0, C // 2)
        prep_v_amax(bufs, tl)
        prep_v_scale_pre(tl)
        prep_v_quant(bufs, tl, pool=False)
        prep_v_aug(tl)

        def k_part2(bufs=bufs, tl=tl):
            prep_k_chain(bufs, tl, TH2, T)

        def kT_part2(tl=tl):
            transpose_group("kq_h", "kqT", "kqT", "sync", tl, C // 2, C)

        def q_part2(bufs=bufs, tl=tl):
            prep_q_chain(bufs, tl, TH2, T)

        def qT_part2(tl=tl):
            transpose_group("qcs_h", "qcsT", "qcsT", "gpsimd", tl, C // 2, C)

        h0_slots0 = {0: [k_part2], 2: [kT_part2], 4: [q_part2],
                     6: [qT_part2]}

        prev_chunks1 = None    # epilogue closure: half1 of previous head
        prev_store = None
        for h in range(heads):
            has_next = h + 1 < heads
            out_sb = osb.tile([128, T, D], f32, tag="out_sb")
            slots0 = dict(h0_slots0) if h == 0 else {}
            h0_slots0 = {}
            if prev_chunks1 is not None:
                slots0.setdefault(3, []).append(prev_chunks1)
                slots0.setdefault(3, []).append(prev_store)
            tl_n = {}
            if has_next:
                bufs_n = load(h + 1)

                def s_cast(bufs_n=bufs_n, tl_n=tl_n):
                    prep_cast_k(bufs_n, tl_n)

                def s_mean(tl_n=tl_n):
                    mean_pe(tl_n)
                    prep_mean_red(tl_n)

                def s_kchain(bufs_n=bufs_n, tl_n=tl_n):
                    prep_k_chain(bufs_n, tl_n)

                def s_qchain(bufs_n=bufs_n, tl_n=tl_n):
                    prep_q_chain(bufs_n, tl_n)

                def s_vamax(bufs_n=bufs_n, tl_n=tl_n):
                    prep_v_amax(bufs_n, tl_n)

                def s_alloc(tl_n=tl_n):
                    alloc_padded("kqT", "kqT", tl_n)
                    alloc_padded("qcsT", "qcsT", tl_n)

                slots0.setdefault(1, []).append(s_cast)
                slots0.setdefault(2, []).append(s_mean)
                slots0.setdefault(0, []).append(s_alloc)
                slots0.setdefault(4, []).append(s_kchain)
                slots0.setdefault(7, []).append(s_qchain)
                slots0.setdefault(10, []).append(s_vamax)
            outT0 = half_loop(h, tl, 0, slots0)

            def store_half(h=h, out_sb=out_sb):
                def _st(half):
                    def go():
                        nc.sync.dma_start(
                            out=o_d[h, half * NQH:(half + 1) * NQH, :]
                            .rearrange("(t p) d -> p t d", p=128),
                            in_=out_sb[:, half * (T // 2):(half + 1) * (T // 2), :])
                    return go
                return _st
            st_mk = store_half()
            slots1 = {}
            slots1[3] = [epilogue_half(outT0, out_sb, 0)]
            slots1[8] = [st_mk(0)]
            if has_next:
                def s_kqT(tl_n=tl_n):
                    transpose_group("kq_h", "kqT", "kqT", "sync", tl_n)

                def s_qcsT(tl_n=tl_n):
                    transpose_group("qcs_h", "qcsT", "qcsT", "sync", tl_n)

                def s_vpre(tl_n=tl_n):
                    prep_v_scale_pre(tl_n)

                def s_vquant(bufs_n=bufs_n, tl_n=tl_n):
                    prep_v_quant(bufs_n, tl_n, pool=True)

                def s_vaug(tl_n=tl_n):
                    prep_v_aug(tl_n)

                slots1[1] = [s_kqT]
                slots1[2] = [s_vpre]
                slots1[4] = [s_vquant]
                slots1[6] = [s_qcsT]
                slots1[11] = [s_vaug]
            outT1 = half_loop(h, tl, 1, slots1)
            prev_chunks1 = epilogue_half(outT1, out_sb, 1)
            prev_store = st_mk(1)
            if has_next:
                tl = tl_n
        # last head's half-1 epilogue tail
        prev_chunks1()
        prev_store()
    return nc


_CACHED = {}


def _get_nc():
    if "nc" not in _CACHED:
        from concourse import bacc

        nc = bacc.Bacc("TRN2", target_bir_lowering=False, debug=False)
        build_attention(nc)
        nc.compile()
        _CACHED["nc"] = nc
    return _CACHED["nc"]


def kernel(q: np.ndarray, k: np.ndarray, v: np.ndarray) -> np.ndarray:
    from concourse.bass_utils import run_bass_kernel_spmd

    nc = _get_nc()
    qf = np.ascontiguousarray(np.asarray(q, dtype=np.float32).reshape(B * H, N, D))
    kf = np.ascontiguousarray(np.asarray(k, dtype=np.float32).reshape(B * H, N, D))
    vf = np.ascontiguousarray(np.asarray(v, dtype=np.float32).reshape(B * H, N, D))
    hpc = HEADS_PER_CORE
    in_maps = [
        {"q": qf[c * hpc:(c + 1) * hpc],
         "k": kf[c * hpc:(c + 1) * hpc],
         "v": vf[c * hpc:(c + 1) * hpc]}
        for c in range(NCORES)
    ]
    res = run_bass_kernel_spmd(nc, in_maps, core_ids=list(range(NCORES)))
    out = np.concatenate([np.asarray(r["out"]) for r in res.results], axis=0)
    return out.reshape(B, H, N, D).astype(np.float32)


# revision 16
# speedup vs baseline: 1.2229x; 1.0179x over previous
"""Trainium2 Bass kernel for Ac4kAttentionOp (int8 q/k + fp8e4m3 v quantized attention).

Shapes: q,k,v [B=2, H=16, N=2048, D=64] fp32 -> out [2,16,2048,64] fp32.
Sharding: 32 (B,H) heads split 4-per-core across 8 NeuronCores; no collectives.

Math (mirrors the reference exactly up to fp32 rounding order):
  k <- k - mean_N(k)
  qq = round(q / sf_q), sf_q = max(amax_D(q)/127, eps)      (per token)
  kq = round(k / sf_k), sf_k = max(amax_D(k)/127, eps)      (per token)
  vq = fp8e4m3(v / sf_v), sf_v = max(amax_N(v)/(448/2.25), eps)  (per channel)
  s^T[m,nq] = sum_d kq[m,d] * (qq[nq,d]*sf_q[nq]*sm) ;  p^T = exp(sf_k[m] * s^T)
  outT[d,nq] = sum_m vq[m,d] * p^T[m,nq] ; denom = ones-column of vq_aug
  out[nq,d] = outT[d,nq] * sf_v[d] / denom[nq]

Performance structure:
  - ACT (exp over all N^2 scores) is the throughput floor (~133us busy/core);
    everything else is arranged to keep it fed back-to-back.
  - All main-loop matmuls (QK and PV) use 128-row fp16 stationaries: kqT/qcsT
    are zero-padded from 64 to 128 contraction rows.  Stationary row-count
    changes between consecutive matmuls serialize LDWEIGHTS (no weight
    preload) and the resulting bubbles pin the PE at its 1.2GHz mid p-state;
    uniform row counts let the PE ramp to 2.4GHz and stay there.
  - Lookahead emission: QK(mt+2)/exp(mt+2) are emitted before PV(mt) so the
    in-order PE queue always has runnable work while ACT computes exp(mt).
  - Per-head prep (quant, DVE) runs during the previous head's half-0 loop;
    prep PE transposes and epilogue transposes are injected as grouped slots
    inside the main loops (ACT's ~2us backlog of queued exps covers the PE
    pause).  Head 0's k/q chains are split into token halves so the first
    QK can issue after roughly half the quant latency.
  - q/v loads and the qcsT parity-split DMAs issue from the Pool (GpSimd)
    DMA queue so transfers overlap the sync-queue ones.
"""
import math
from contextlib import ExitStack

import numpy as np

import concourse.bass as bass
import concourse.tile as tile
from concourse import mybir
from concourse.masks import make_identity

B, H, N, D = 2, 16, 2048, 64
NCORES = 8
HEADS_PER_CORE = (B * H) // NCORES          # 4
SM_SCALE = 1.0 / math.sqrt(D)               # 0.125 (exact power of 2)
MAGIC = 12582912.0                          # 1.5*2^23: fp32 RNE integer round
INT8_MAX = 127.0
F8_AMAX_DIV = float(np.float32(448.0) / np.float32(2.25))  # FP8_MAX / MAX_SCALE
EPS = 1e-8

f32 = mybir.dt.float32
f16 = mybir.dt.float16
f8e4 = mybir.dt.float8e4
ALU = mybir.AluOpType
ACTF = mybir.ActivationFunctionType


def _bc(t: bass.AP, dims, off: int = 0) -> bass.AP:
    """Build a broadcast/restrided view of a tile AP (off in elements)."""
    return bass.AP(tensor=t.tensor, offset=t.offset + off, ap=dims)


def build_attention(nc: bass.Bass, heads: int = HEADS_PER_CORE, n: int = N,
                    bench_loops: int = 0):
    T = n // 128          # token tiles per head
    C = T // 2            # 128-wide transpose chunks
    NQH = n // 2          # query-half width (PSUM budget)
    q_d = nc.dram_tensor("q", [heads, n, D], f32, kind="ExternalInput").ap()
    k_d = nc.dram_tensor("k", [heads, n, D], f32, kind="ExternalInput").ap()
    v_d = nc.dram_tensor("v", [heads, n, D], f32, kind="ExternalInput").ap()
    o_d = nc.dram_tensor("out", [heads, n, D], f32, kind="ExternalOutput").ap()

    with tile.TileContext(nc) as tc, ExitStack() as ctx:
        singles = ctx.enter_context(tc.tile_pool(name="singles", bufs=1))
        loads = ctx.enter_context(tc.tile_pool(name="loads", bufs=2))
        work = ctx.enter_context(tc.tile_pool(name="work", bufs=2))
        scales = ctx.enter_context(tc.tile_pool(name="scales", bufs=2))
        small = ctx.enter_context(tc.tile_pool(name="small", bufs=4))
        opnds = ctx.enter_context(tc.tile_pool(name="opnds", bufs=2))
        pbuf = ctx.enter_context(tc.tile_pool(name="pbuf", bufs=4))
        obuf = ctx.enter_context(tc.tile_pool(name="obuf", bufs=2))
        ostore = ctx.enter_context(tc.tile_pool(name="ostore", bufs=4))
        osb = ctx.enter_context(tc.tile_pool(name="osb", bufs=2))
        ps_s = ctx.enter_context(tc.tile_pool(name="ps_s", bufs=2, space="PSUM"))
        ps_o = ctx.enter_context(tc.tile_pool(name="ps_o", bufs=1, space="PSUM"))
        ps_t = ctx.enter_context(tc.tile_pool(name="ps_t", bufs=2, space="PSUM"))

        ident_f = singles.tile([128, 128], f32)
        make_identity(nc, ident_f)
        ident_h = singles.tile([128, 128], f16)
        make_identity(nc, ident_h)
        ones_row = singles.tile([1, 128], f32)
        nc.gpsimd.memset(ones_row, 1.0)
        # constant [128,128] of 1/n in f16 (2^-11, exact): k-mean matmul weights
        invn_h = singles.tile([128, 128], f16)
        nc.gpsimd.memset(invn_h, 1.0 / n)

        if bench_loops:
            ctx.enter_context(tc.For_i(0, bench_loops, 1))

        # warm the ACT exp table before the first real exp
        warm = singles.tile([1, 1], f32)
        nc.gpsimd.memset(warm, 0.0)
        nc.scalar.activation(warm, warm, ACTF.Exp)

        def load(h):
            """k on the sync DMA queue; q and v on the gpsimd queue so the
            three transfers run in parallel.  k first: it heads the longest
            prep chain (mean -> sub -> quant -> transpose)."""
            k_sb = loads.tile([128, T, D], f32, tag="k_sb")
            nc.sync.dma_start(out=k_sb,
                              in_=k_d[h].rearrange("(t p) d -> p t d", p=128))
            q_sb = loads.tile([128, T, D], f32, tag="q_sb")
            nc.gpsimd.dma_start(out=q_sb,
                                in_=q_d[h].rearrange("(t p) d -> p t d", p=128))
            v_sb = loads.tile([128, T, D], f32, tag="v_sb")
            nc.gpsimd.dma_start(out=v_sb,
                                in_=v_d[h].rearrange("(t p) d -> p t d", p=128))
            return q_sb, k_sb, v_sb

        def prep_cast_k(bufs, tl):
            """DVE: k -> f16 (mean-matmul moving operand)."""
            _, k_sb, _ = bufs
            tl["k_h"] = work.tile([128, T, D], f16, tag="k_h", name="k_h")
            nc.vector.tensor_copy(tl["k_h"], k_sb)

        def mean_pe(tl):
            """PE: column-mean partials via 1/n matmul (same shape family as
            the main-loop matmuls: [128,128] f16 stationary)."""
            mean_ps = ps_s.tile([128, NQH], f32, tag="pss")
            half_td = T * D // 2
            nc.tensor.matmul(mean_ps[:, 0:half_td], invn_h,
                             tl["k_h"][:, 0:T // 2, :], start=True, stop=True)
            nc.tensor.matmul(mean_ps[:, half_td:T * D], invn_h,
                             tl["k_h"][:, T // 2:T, :], start=True, stop=True)
            tl["mean_ps"] = mean_ps

        def prep_mean_red(tl):
            """DVE: reduce mean partials over token tiles."""
            meanb = small.tile([128, D], f32, tag="meanb")
            mean_ps = tl["mean_ps"]
            nc.vector.tensor_reduce(
                out=meanb,
                in_=_bc(mean_ps, [mean_ps.ap[0], [1, D], [D, T]]),
                axis=mybir.AxisListType.X, op=ALU.add)
            tl["meanb"] = meanb

        def quant_int8(x_sb, tagpfx, tl, t0, t1):
            """per-token int8 quantize of tiles [t0,t1); scale/stage tiles in
            tl are allocated on the first part, sub-written on later parts."""
            nt = t1 - t0
            key = tagpfx + "amax"
            if key not in tl:
                tl[key] = scales.tile([128, T], f32, tag=key, name=key)
                tl[tagpfx + "sf"] = scales.tile([128, T], f32,
                                                tag=tagpfx + "sf",
                                                name=tagpfx + "sf")
                tl[tagpfx + "rsf"] = scales.tile([128, T], f32,
                                                 tag=tagpfx + "rsf",
                                                 name=tagpfx + "rsf")
                tl[tagpfx + "xq"] = work.tile([128, T, D], f32,
                                              tag=tagpfx + "xq",
                                              name=tagpfx + "xq")
            amax, sf = tl[key], tl[tagpfx + "sf"]
            rsf, xq = tl[tagpfx + "rsf"], tl[tagpfx + "xq"]
            nc.vector.tensor_reduce(out=amax[:, t0:t1], in_=x_sb[:, t0:t1, :],
                                    axis=mybir.AxisListType.X, op=ALU.max,
                                    apply_absolute_value=True)
            nc.vector.tensor_scalar(out=sf[:, t0:t1], in0=amax[:, t0:t1],
                                    scalar1=1.0 / INT8_MAX, scalar2=EPS,
                                    op0=ALU.mult, op1=ALU.max)
            nc.vector.reciprocal(rsf[:, t0:t1], sf[:, t0:t1])
            nc.vector.tensor_mul(
                xq[:, t0:t1, :], x_sb[:, t0:t1, :],
                _bc(rsf, [rsf.ap[0], [1, nt], [0, D]], off=t0))
            # RNE integer round: (x + MAGIC) - MAGIC
            nc.vector.tensor_scalar(out=xq[:, t0:t1, :], in0=xq[:, t0:t1, :],
                                    scalar1=MAGIC, scalar2=MAGIC,
                                    op0=ALU.add, op1=ALU.subtract)

        def prep_k_chain(bufs, tl, t0=0, t1=None):
            """DVE: mean-sub + int8 quant + f16 cast for k tiles [t0,t1)."""
            _, k_sb, _ = bufs
            t1 = T if t1 is None else t1
            nt = t1 - t0
            if "ks" not in tl:
                tl["ks"] = work.tile([128, T, D], f32, tag="ks", name="ks")
                tl["kq_h"] = work.tile([128, T, D], f16, tag="kq_h", name="kq_h")
            meanb = tl["meanb"]
            nc.vector.tensor_sub(tl["ks"][:, t0:t1, :], k_sb[:, t0:t1, :],
                                 _bc(meanb, [meanb.ap[0], [0, nt], [1, D]]))
            quant_int8(tl["ks"], "k", tl, t0, t1)
            nc.vector.tensor_copy(tl["kq_h"][:, t0:t1, :],
                                  tl["kxq"][:, t0:t1, :])

        def prep_q_chain(bufs, tl, t0=0, t1=None):
            """DVE: int8 quant + fold sf_q*sm + f16 cast for q tiles."""
            q_sb, _, _ = bufs
            t1 = T if t1 is None else t1
            nt = t1 - t0
            if "qcs_h" not in tl:
                tl["csfq"] = scales.tile([128, T], f32, tag="csfq", name="csfq")
                tl["qcs"] = work.tile([128, T, D], f32, tag="qcs", name="qcs")
                tl["qcs_h"] = work.tile([128, T, D], f16, tag="qcs_h", name="qcs_h")
            quant_int8(q_sb, "q", tl, t0, t1)
            csfq = tl["csfq"]
            nc.vector.tensor_scalar_mul(csfq[:, t0:t1], tl["qsf"][:, t0:t1],
                                        SM_SCALE)
            nc.vector.tensor_mul(
                tl["qcs"][:, t0:t1, :], tl["qxq"][:, t0:t1, :],
                _bc(csfq, [csfq.ap[0], [1, nt], [0, D]], off=t0))
            nc.vector.tensor_copy(tl["qcs_h"][:, t0:t1, :],
                                  tl["qcs"][:, t0:t1, :])

        def prep_v_amax(bufs, tl):
            """DVE: per-channel |v| max partials (channel-major view)."""
            _, _, v_sb = bufs
            amax_vp = work.tile([128, D], f32, tag="amax_vp")
            nc.vector.tensor_reduce(
                out=amax_vp,
                in_=_bc(v_sb, [v_sb.ap[0], [1, D], [D, T]]),
                axis=mybir.AxisListType.X, op=ALU.max,
                apply_absolute_value=True)
            tl["amax_vp"] = amax_vp

        def alloc_padded(dst_key, tag, tl):
            """Allocate a zero-padded transposed operand + its stack buffer;
            the Pool memset of the pad runs early, off the critical path."""
            tl[dst_key] = opnds.tile([128, T, 128], f16, tag=tag, name=tag)
            nc.gpsimd.memset(tl[dst_key][64:128, :, :], 0.0)
            tl[dst_key + "_st"] = work.tile([128, C, 128], f16,
                                            tag=tag + "_st",
                                            name=tag + "_st")

        def transpose_group(src_key, dst_key, tag, queue, tl, c0=0, c1=None):
            """PE chunk transposes (parity-stacked via DVE) of chunks [c0,c1)
            then two strided parity-split DMAs into the top half of the
            zero-padded [128,(T,128)] operand."""
            c1 = C if c1 is None else c1
            if dst_key not in tl:
                alloc_padded(dst_key, tag, tl)
            dstT, stk = tl[dst_key], tl[dst_key + "_st"]
            x_h = tl[src_key]
            for c in range(c0, c1):
                tp = ps_t.tile([128, 128], f16, tag="pst")
                nc.tensor.transpose(tp, x_h[:, 2 * c:2 * c + 2, :], ident_h)
                nc.vector.tensor_copy(stk[:, c, :], tp)
            eng = nc.sync if queue == "sync" else nc.gpsimd
            d64 = dstT[0:64]
            nci = c1 - c0
            eng.dma_start(
                out=_bc(d64, [d64.ap[0], [2 * 128, nci], [1, 128]],
                        off=c0 * 256),
                in_=stk[0:64, c0:c1, :])
            eng.dma_start(
                out=_bc(d64, [d64.ap[0], [2 * 128, nci], [1, 128]],
                        off=c0 * 256 + 128),
                in_=stk[64:128, c0:c1, :])

        def prep_v_scale_pre(tl):
            """PE transpose of amax partials + DVE scale math (all small)."""
            vt_ps = ps_t.tile([D, 128], f32, tag="pst")
            nc.tensor.transpose(vt_ps, tl["amax_vp"], ident_f)
            amax_vT = scales.tile([D, 1], f32, tag="amax_vT")
            nc.vector.tensor_reduce(out=amax_vT, in_=vt_ps,
                                    axis=mybir.AxisListType.X, op=ALU.max)
            sf_vT = scales.tile([D, 1], f32, tag="sf_vT")
            nc.vector.tensor_scalar(out=sf_vT, in0=amax_vT,
                                    scalar1=1.0 / F8_AMAX_DIV, scalar2=EPS,
                                    op0=ALU.mult, op1=ALU.max)
            rsf_vT = scales.tile([D, 1], f32, tag="rsf_vT")
            nc.vector.reciprocal(rsf_vT, sf_vT)
            sfv65 = scales.tile([65, 1], f32, tag="sfv65")
            nc.gpsimd.memset(sfv65, 1.0)
            nc.vector.tensor_copy(sfv65[0:D, :], sf_vT)
            rsf_row = small.tile([1, D], f32, tag="rsf_row")
            nc.sync.dma_start(out=rsf_row, in_=rsf_vT)
            rsf_bps = ps_t.tile([128, D], f32, tag="pst")
            nc.tensor.matmul(rsf_bps, ones_row, rsf_row, start=True, stop=True)
            rsf_b = small.tile([128, D], f32, tag="rsf_b")
            nc.vector.tensor_copy(rsf_b, rsf_bps)
            tl["rsf_b"] = rsf_b
            tl["sfv65"] = sfv65

        def prep_v_quant(bufs, tl, pool=True):
            """fp8 quantize v (scale-multiply + cast to the f8e4 grid).  The
            two big elementwise ops go to Pool in steady state (keeps DVE
            free so the o_ps scale fires promptly)."""
            _, _, v_sb = bufs
            rsf_b = tl["rsf_b"]
            eng = nc.gpsimd if pool else nc.vector
            vq_pre = work.tile([128, T, D], f32, tag="vq_pre")
            eng.tensor_mul(vq_pre, v_sb,
                           _bc(rsf_b, [rsf_b.ap[0], [0, T], [1, D]]))
            vq_f8 = work.tile([128, T, D], f8e4, tag="vq_f8")
            eng.tensor_copy(vq_f8, vq_pre)
            tl["vq_f8"] = vq_f8

        def prep_v_aug(tl):
            """DVE: f8 grid values -> f16 PV stationary + ones column."""
            vq_aug = opnds.tile([128, T, D + 1], f16, tag="vq_aug")
            nc.vector.tensor_copy(vq_aug[:, :, 0:D], tl["vq_f8"])
            nc.gpsimd.memset(vq_aug[:, :, D:D + 1], 1.0)
            tl["vq_aug"] = vq_aug

        def half_loop(h, tl, half, slots=None):
            """Main QK->exp->PV loop for one query half (NQH queries).
            Lookahead: QK(mt+2)/exp(mt+2) emitted before PV(mt).
            slots: {mt: [closure,...]} run after qk_exp(mt+2) is emitted."""
            slots = slots or {}
            kqT, qcsT, vq_aug = tl["kqT"], tl["qcsT"], tl["vq_aug"]
            sf_k = tl["ksf"]
            TH = T // 2

            def qk_exp(mt):
                s_ps = ps_s.tile([128, NQH], f32, tag="pss")
                for j in range(NQH // 512):
                    rhs = qcsT[:, half * TH + 4 * j:half * TH + 4 * (j + 1), :]
                    nc.tensor.matmul(s_ps[:, j * 512:(j + 1) * 512],
                                     kqT[:, mt, :], rhs, start=True, stop=True)
                p_sb = pbuf.tile([128, NQH], f16, tag="p_sb")
                nc.scalar.activation(p_sb, s_ps, ACTF.Exp,
                                     scale=sf_k[:, mt:mt + 1])
                return p_sb

            o_ps = ps_o.tile([65, NQH], f32, tag="pso")
            ps = [qk_exp(0), qk_exp(1)]
            for mt in range(T):
                if mt + 2 < T:
                    ps.append(qk_exp(mt + 2))
                for fn in slots.get(mt, ()):
                    fn()
                p_sb = ps[mt]
                for j in range(NQH // 512):
                    nc.tensor.matmul(
                        o_ps[:, j * 512:(j + 1) * 512],
                        vq_aug[:, mt, :],
                        p_sb[:, j * 512:(j + 1) * 512],
                        start=(mt == 0), stop=(mt == T - 1))
            # scale by per-channel v scale, park in SBUF (frees the psum bank)
            outT_sb = obuf.tile([65, NQH], f32, tag="outT")
            nc.vector.tensor_scalar_mul(outT_sb, o_ps, tl["sfv65"][:, 0:1])
            return outT_sb

        def epilogue_half(outT_sb, out_sb, half, act_mul=False):
            """Out-transposes + denominator divide for one query half.
            Returns one closure per 128-query chunk so the caller can spread
            them thin across main-loop slots (each is a short PE pause).
            act_mul: do the divide-multiply on ACT (only when ACT is idle,
            i.e. the final tail)."""
            def mk(c):
                def chunk():
                    tp2 = ps_t.tile([128, 65], f32, tag="pst")
                    nc.tensor.transpose(tp2, outT_sb[:, c * 128:(c + 1) * 128],
                                        ident_f[0:65, 0:65])
                    rec = ostore.tile([128, 1], f32, tag="rec")
                    nc.vector.reciprocal(rec, tp2[:, D:D + 1])
                    if act_mul:
                        nc.scalar.activation(
                            out_sb[:, half * (T // 2) + c, :], tp2[:, 0:D],
                            ACTF.Copy, scale=rec)
                    else:
                        nc.vector.tensor_mul(
                            out_sb[:, half * (T // 2) + c, :], tp2[:, 0:D],
                            _bc(rec, [rec.ap[0], [0, D]]))
                return chunk
            return [mk(c) for c in range(NQH // 128)]

        # ---- head pipeline ----
        # Emission schedule (engines execute in emission order, per engine):
        #  half0(h): slot0 alloc pads (h+1, Pool) ; slot1 k cast (h+1, DVE);
        #            slot2 mean (h+1, PE+DVE); slots3-10 one epilogue chunk
        #            of (h-1) half1 each; slot4 k quant chain (h+1, DVE);
        #            slot8 q quant chain; slot11 v amax; slot12 store(h-1).
        #  half1(h): slots1,3 kqT transposes (h+1, 4 chunks each + DMA);
        #            slot2 v scale prefix; slot4 v fp8 quant (Pool);
        #            slots5,7 qcsT transposes; slots8-15 one epilogue chunk
        #            of (h) half0 each; slot13 vq_aug build (DVE).
        # Head 0 is prepped serially (k/q chains split into token halves so
        # the first QK issues after roughly half the quant latency); head 1's
        # prep shifts one slot-group later because head 0's DVE is saturated.
        tl = {}
        bufs = load(0)
        prep_cast_k(bufs, tl)
        mean_pe(tl)
        prep_mean_red(tl)
        TH2 = T // 2
        prep_k_chain(bufs, tl, 0, TH2)
        transpose_group("kq_h", "kqT", "kqT", "sync", tl, 0, C // 2)
        prep_q_chain(bufs, tl, 0, TH2)
        transpose_group("qcs_h", "qcsT", "qcsT", "gpsimd", tl, 0, C // 2)
        prep_v_amax(bufs, tl)
        prep_v_scale_pre(tl)
        prep_v_quant(bufs, tl, pool=False)
        prep_v_aug(tl)

        def k_part2(bufs=bufs, tl=tl):
            prep_k_chain(bufs, tl, TH2, T)

        def kT_part2(tl=tl):
            transpose_group("kq_h", "kqT", "kqT", "sync", tl, C // 2, C)

        def q_part2(bufs=bufs, tl=tl):
            prep_q_chain(bufs, tl, TH2, T)

        def qT_part2(tl=tl):
            transpose_group("qcs_h", "qcsT", "qcsT", "gpsimd", tl, C // 2, C)

        h0_slots0 = {0: [k_part2], 2: [kT_part2], 4: [q_part2],
                     6: [qT_part2]}

        prev_chunks1 = None    # per-chunk epilogue closures: prev head half1
        prev_store = None
        for h in range(heads):
            has_next = h + 1 < heads
            late = 2 if h == 0 else 0    # shift next-head prep on head 0
            out_sb = osb.tile([128, T, D], f32, tag="out_sb")
            slots0 = dict(h0_slots0) if h == 0 else {}
            h0_slots0 = {}
            if prev_chunks1 is not None:
                for i, ck in enumerate(prev_chunks1):
                    slots0.setdefault(3 + i, []).append(ck)
                slots0.setdefault(12, []).append(prev_store)
            tl_n = {}
            if has_next:
                bufs_n = load(h + 1)

                def s_cast(bufs_n=bufs_n, tl_n=tl_n):
                    prep_cast_k(bufs_n, tl_n)

                def s_mean(tl_n=tl_n):
                    mean_pe(tl_n)
                    prep_mean_red(tl_n)

                def s_kchain(bufs_n=bufs_n, tl_n=tl_n):
                    prep_k_chain(bufs_n, tl_n)

                def s_qchain(bufs_n=bufs_n, tl_n=tl_n):
                    prep_q_chain(bufs_n, tl_n)

                def s_vamax(bufs_n=bufs_n, tl_n=tl_n):
                    prep_v_amax(bufs_n, tl_n)

                def s_alloc(tl_n=tl_n):
                    alloc_padded("kqT", "kqT", tl_n)
                    alloc_padded("qcsT", "qcsT", tl_n)

                slots0.setdefault(0, []).append(s_alloc)
                slots0.setdefault(1, []).append(s_cast)
                slots0.setdefault(2 + late, []).append(s_mean)
                slots0.setdefault(4 + late, []).append(s_kchain)
                slots0.setdefault(8 + late, []).append(s_qchain)
                slots0.setdefault(11 + late, []).append(s_vamax)
            outT0 = half_loop(h, tl, 0, slots0)

            def mk_store(h=h, out_sb=out_sb):
                def go():
                    nc.sync.dma_start(
                        out=o_d[h].rearrange("(t p) d -> p t d", p=128),
                        in_=out_sb)
                return go
            slots1 = {}
            for i, ck in enumerate(epilogue_half(outT0, out_sb, 0)):
                slots1.setdefault(8 + i if i < 7 else 15, []).append(ck)
            if has_next:
                def s_kqT_a(tl_n=tl_n):
                    transpose_group("kq_h", "kqT", "kqT", "sync", tl_n,
                                    0, C // 2)

                def s_kqT_b(tl_n=tl_n):
                    transpose_group("kq_h", "kqT", "kqT", "sync", tl_n,
                                    C // 2, C)

                def s_qcsT_a(tl_n=tl_n):
                    transpose_group("qcs_h", "qcsT", "qcsT", "sync", tl_n,
                                    0, C // 2)

                def s_qcsT_b(tl_n=tl_n):
                    transpose_group("qcs_h", "qcsT", "qcsT", "sync", tl_n,
                                    C // 2, C)

                def s_vpre(tl_n=tl_n):
                    prep_v_scale_pre(tl_n)

                def s_vquant(bufs_n=bufs_n, tl_n=tl_n):
                    prep_v_quant(bufs_n, tl_n, pool=True)

                def s_vaug(tl_n=tl_n):
                    prep_v_aug(tl_n)

                slots1.setdefault(1 + late, []).append(s_kqT_a)
                slots1.setdefault(2 + late, []).append(s_vpre)
                slots1.setdefault(3 + late, []).append(s_kqT_b)
                slots1.setdefault(4 + late, []).append(s_vquant)
                slots1.setdefault(5 + late, []).append(s_qcsT_a)
                slots1.setdefault(7 + late, []).append(s_qcsT_b)
                slots1.setdefault(13, []).append(s_vaug)
            outT1 = half_loop(h, tl, 1, slots1)
            prev_chunks1 = epilogue_half(outT1, out_sb, 1,
                                         act_mul=not has_next)
            prev_store = mk_store()
            if has_next:
                tl = tl_n
        # last head's half-1 epilogue tail (ACT is idle here: use it for the
        # divide so the chunk round-trip is shorter)
        for ck in prev_chunks1:
            ck()
        prev_store()
    return nc


_CACHED = {}


def _get_nc():
    if "nc" not in _CACHED:
        from concourse import bacc

        nc = bacc.Bacc("TRN2", target_bir_lowering=False, debug=False)
        build_attention(nc)
        nc.compile()
        _CACHED["nc"] = nc
    return _CACHED["nc"]


def kernel(q: np.ndarray, k: np.ndarray, v: np.ndarray) -> np.ndarray:
    from concourse.bass_utils import run_bass_kernel_spmd

    nc = _get_nc()
    qf = np.ascontiguousarray(np.asarray(q, dtype=np.float32).reshape(B * H, N, D))
    kf = np.ascontiguousarray(np.asarray(k, dtype=np.float32).reshape(B * H, N, D))
    vf = np.ascontiguousarray(np.asarray(v, dtype=np.float32).reshape(B * H, N, D))
    hpc = HEADS_PER_CORE
    in_maps = [
        {"q": qf[c * hpc:(c + 1) * hpc],
         "k": kf[c * hpc:(c + 1) * hpc],
         "v": vf[c * hpc:(c + 1) * hpc]}
        for c in range(NCORES)
    ]
    res = run_bass_kernel_spmd(nc, in_maps, core_ids=list(range(NCORES)))
    out = np.concatenate([np.asarray(r["out"]) for r in res.results], axis=0)
    return out.reshape(B, H, N, D).astype(np.float32)


# revision 23
# speedup vs baseline: 1.2271x; 1.0034x over previous
"""Trainium2 Bass kernel for Ac4kAttentionOp (int8 q/k + fp8e4m3 v quantized attention).

Shapes: q,k,v [B=2, H=16, N=2048, D=64] fp32 -> out [2,16,2048,64] fp32.
Sharding: 32 (B,H) heads split 4-per-core across 8 NeuronCores; no collectives.

Math (mirrors the reference exactly up to fp32 rounding order):
  k <- k - mean_N(k)
  qq = round(q / sf_q), sf_q = max(amax_D(q)/127, eps)      (per token)
  kq = round(k / sf_k), sf_k = max(amax_D(k)/127, eps)      (per token)
  vq = fp8e4m3(v / sf_v), sf_v = max(amax_N(v)/(448/2.25), eps)  (per channel)
  s^T[m,nq] = sum_d kq[m,d] * (qq[nq,d]*sf_q[nq]*sm) ;  p^T = exp(sf_k[m] * s^T)
  outT[d,nq] = sum_m vq[m,d] * p^T[m,nq] ; denom = ones-column of vq_aug
  out[nq,d] = outT[d,nq] * sf_v[d] / denom[nq]

Performance structure:
  - ACT (exp over all N^2 scores) is the throughput floor (~133us busy/core);
    everything else is arranged to keep it fed back-to-back.
  - All main-loop matmuls (QK and PV) use 128-row fp16 stationaries: kqT/qcsT
    are zero-padded from 64 to 128 contraction rows.  Stationary row-count
    changes between consecutive matmuls serialize LDWEIGHTS (no weight
    preload) and the resulting bubbles pin the PE at its 1.2GHz mid p-state;
    uniform row counts let the PE ramp to 2.4GHz and stay there.
  - Lookahead emission: QK(mt+2)/exp(mt+2) are emitted before PV(mt) so the
    in-order PE queue always has runnable work while ACT computes exp(mt).
  - Per-head prep (quant, DVE) runs during the previous head's half-0 loop;
    prep PE transposes and epilogue transposes are injected as grouped slots
    inside the main loops (ACT's ~2us backlog of queued exps covers the PE
    pause).  Head 0's k/q chains are split into token halves so the first
    QK can issue after roughly half the quant latency.
  - q/v loads and the qcsT parity-split DMAs issue from the Pool (GpSimd)
    DMA queue so transfers overlap the sync-queue ones.
"""
import math
from contextlib import ExitStack

import numpy as np

import concourse.bass as bass
import concourse.tile as tile
from concourse import mybir
from concourse.masks import make_identity

B, H, N, D = 2, 16, 2048, 64
NCORES = 8
HEADS_PER_CORE = (B * H) // NCORES          # 4
SM_SCALE = 1.0 / math.sqrt(D)               # 0.125 (exact power of 2)
MAGIC = 12582912.0                          # 1.5*2^23: fp32 RNE integer round
INT8_MAX = 127.0
F8_AMAX_DIV = float(np.float32(448.0) / np.float32(2.25))  # FP8_MAX / MAX_SCALE
EPS = 1e-8

f32 = mybir.dt.float32
f16 = mybir.dt.float16
f8e4 = mybir.dt.float8e4
ALU = mybir.AluOpType
ACTF = mybir.ActivationFunctionType


def _bc(t: bass.AP, dims, off: int = 0) -> bass.AP:
    """Build a broadcast/restrided view of a tile AP (off in elements)."""
    return bass.AP(tensor=t.tensor, offset=t.offset + off, ap=dims)


def build_attention(nc: bass.Bass, heads: int = HEADS_PER_CORE, n: int = N,
                    bench_loops: int = 0):
    T = n // 128          # token tiles per head
    C = T // 2            # 128-wide transpose chunks
    NQH = n // 2          # query-half width (PSUM budget)
    q_d = nc.dram_tensor("q", [heads, n, D], f32, kind="ExternalInput").ap()
    k_d = nc.dram_tensor("k", [heads, n, D], f32, kind="ExternalInput").ap()
    v_d = nc.dram_tensor("v", [heads, n, D], f32, kind="ExternalInput").ap()
    o_d = nc.dram_tensor("out", [heads, n, D], f32, kind="ExternalOutput").ap()

    with tile.TileContext(nc) as tc, ExitStack() as ctx:
        singles = ctx.enter_context(tc.tile_pool(name="singles", bufs=1))
        loads = ctx.enter_context(tc.tile_pool(name="loads", bufs=2))
        work = ctx.enter_context(tc.tile_pool(name="work", bufs=2))
        scales = ctx.enter_context(tc.tile_pool(name="scales", bufs=2))
        small = ctx.enter_context(tc.tile_pool(name="small", bufs=4))
        opnds = ctx.enter_context(tc.tile_pool(name="opnds", bufs=2))
        pbuf = ctx.enter_context(tc.tile_pool(name="pbuf", bufs=4))
        obuf = ctx.enter_context(tc.tile_pool(name="obuf", bufs=2))
        ostore = ctx.enter_context(tc.tile_pool(name="ostore", bufs=4))
        osb = ctx.enter_context(tc.tile_pool(name="osb", bufs=2))
        ps_s = ctx.enter_context(tc.tile_pool(name="ps_s", bufs=2, space="PSUM"))
        ps_o = ctx.enter_context(tc.tile_pool(name="ps_o", bufs=2, space="PSUM"))
        ps_t = ctx.enter_context(tc.tile_pool(name="ps_t", bufs=2, space="PSUM"))

        ident_f = singles.tile([128, 128], f32)
        make_identity(nc, ident_f)
        ident_h = singles.tile([128, 128], f16)
        make_identity(nc, ident_h)
        ones_row = singles.tile([1, 128], f32)
        nc.gpsimd.memset(ones_row, 1.0)
        # constant [128,128] of 1/n in f16 (2^-11, exact): k-mean matmul weights
        invn_h = singles.tile([128, 128], f16)
        nc.gpsimd.memset(invn_h, 1.0 / n)

        if bench_loops:
            ctx.enter_context(tc.For_i(0, bench_loops, 1))

        # warm the ACT exp table before the first real exp
        warm = singles.tile([1, 1], f32)
        nc.gpsimd.memset(warm, 0.0)
        nc.scalar.activation(warm, warm, ACTF.Exp)

        def load(h):
            """k on the sync DMA queue; q and v on the gpsimd queue so the
            three transfers run in parallel.  k first: it heads the longest
            prep chain (mean -> sub -> quant -> transpose)."""
            k_sb = loads.tile([128, T, D], f32, tag="k_sb")
            nc.sync.dma_start(out=k_sb,
                              in_=k_d[h].rearrange("(t p) d -> p t d", p=128))
            q_sb = loads.tile([128, T, D], f32, tag="q_sb")
            nc.gpsimd.dma_start(out=q_sb,
                                in_=q_d[h].rearrange("(t p) d -> p t d", p=128))
            v_sb = loads.tile([128, T, D], f32, tag="v_sb")
            nc.gpsimd.dma_start(out=v_sb,
                                in_=v_d[h].rearrange("(t p) d -> p t d", p=128))
            return q_sb, k_sb, v_sb

        def prep_cast_k(bufs, tl):
            """DVE: k -> f16 (mean-matmul moving operand)."""
            _, k_sb, _ = bufs
            tl["k_h"] = work.tile([128, T, D], f16, tag="k_h", name="k_h")
            nc.vector.tensor_copy(tl["k_h"], k_sb)

        def mean_pe(tl):
            """PE: column-mean partials via 1/n matmul (same shape family as
            the main-loop matmuls: [128,128] f16 stationary)."""
            mean_ps = ps_s.tile([128, NQH], f32, tag="pss")
            half_td = T * D // 2
            nc.tensor.matmul(mean_ps[:, 0:half_td], invn_h,
                             tl["k_h"][:, 0:T // 2, :], start=True, stop=True)
            nc.tensor.matmul(mean_ps[:, half_td:T * D], invn_h,
                             tl["k_h"][:, T // 2:T, :], start=True, stop=True)
            tl["mean_ps"] = mean_ps

        def prep_mean_red(tl):
            """DVE: reduce mean partials over token tiles."""
            meanb = small.tile([128, D], f32, tag="meanb")
            mean_ps = tl["mean_ps"]
            nc.vector.tensor_reduce(
                out=meanb,
                in_=_bc(mean_ps, [mean_ps.ap[0], [1, D], [D, T]]),
                axis=mybir.AxisListType.X, op=ALU.add)
            tl["meanb"] = meanb

        def quant_int8(x_sb, tagpfx, tl, t0, t1):
            """per-token int8 quantize of tiles [t0,t1); scale/stage tiles in
            tl are allocated on the first part, sub-written on later parts."""
            nt = t1 - t0
            key = tagpfx + "amax"
            if key not in tl:
                tl[key] = scales.tile([128, T], f32, tag=key, name=key)
                tl[tagpfx + "sf"] = scales.tile([128, T], f32,
                                                tag=tagpfx + "sf",
                                                name=tagpfx + "sf")
                tl[tagpfx + "rsf"] = scales.tile([128, T], f32,
                                                 tag=tagpfx + "rsf",
                                                 name=tagpfx + "rsf")
                tl[tagpfx + "xq"] = work.tile([128, T, D], f32,
                                              tag=tagpfx + "xq",
                                              name=tagpfx + "xq")
            amax, sf = tl[key], tl[tagpfx + "sf"]
            rsf, xq = tl[tagpfx + "rsf"], tl[tagpfx + "xq"]
            nc.vector.tensor_reduce(out=amax[:, t0:t1], in_=x_sb[:, t0:t1, :],
                                    axis=mybir.AxisListType.X, op=ALU.max,
                                    apply_absolute_value=True)
            nc.vector.tensor_scalar(out=sf[:, t0:t1], in0=amax[:, t0:t1],
                                    scalar1=1.0 / INT8_MAX, scalar2=EPS,
                                    op0=ALU.mult, op1=ALU.max)
            nc.vector.reciprocal(rsf[:, t0:t1], sf[:, t0:t1])
            nc.vector.tensor_mul(
                xq[:, t0:t1, :], x_sb[:, t0:t1, :],
                _bc(rsf, [rsf.ap[0], [1, nt], [0, D]], off=t0))
            # RNE integer round: (x + MAGIC) - MAGIC
            nc.vector.tensor_scalar(out=xq[:, t0:t1, :], in0=xq[:, t0:t1, :],
                                    scalar1=MAGIC, scalar2=MAGIC,
                                    op0=ALU.add, op1=ALU.subtract)

        def prep_k_chain(bufs, tl, t0=0, t1=None):
            """DVE: mean-sub + int8 quant + f16 cast for k tiles [t0,t1)."""
            _, k_sb, _ = bufs
            t1 = T if t1 is None else t1
            nt = t1 - t0
            if "ks" not in tl:
                tl["ks"] = work.tile([128, T, D], f32, tag="ks", name="ks")
                tl["kq_h"] = work.tile([128, T, D], f16, tag="kq_h", name="kq_h")
            meanb = tl["meanb"]
            nc.vector.tensor_sub(tl["ks"][:, t0:t1, :], k_sb[:, t0:t1, :],
                                 _bc(meanb, [meanb.ap[0], [0, nt], [1, D]]))
            quant_int8(tl["ks"], "k", tl, t0, t1)
            nc.vector.tensor_copy(tl["kq_h"][:, t0:t1, :],
                                  tl["kxq"][:, t0:t1, :])

        def prep_q_chain(bufs, tl, t0=0, t1=None):
            """DVE: int8 quant + fold sf_q*sm + f16 cast for q tiles."""
            q_sb, _, _ = bufs
            t1 = T if t1 is None else t1
            nt = t1 - t0
            if "qcs_h" not in tl:
                tl["csfq"] = scales.tile([128, T], f32, tag="csfq", name="csfq")
                tl["qcs"] = work.tile([128, T, D], f32, tag="qcs", name="qcs")
                tl["qcs_h"] = work.tile([128, T, D], f16, tag="qcs_h", name="qcs_h")
            quant_int8(q_sb, "q", tl, t0, t1)
            csfq = tl["csfq"]
            nc.vector.tensor_scalar_mul(csfq[:, t0:t1], tl["qsf"][:, t0:t1],
                                        SM_SCALE)
            nc.vector.tensor_mul(
                tl["qcs"][:, t0:t1, :], tl["qxq"][:, t0:t1, :],
                _bc(csfq, [csfq.ap[0], [1, nt], [0, D]], off=t0))
            nc.vector.tensor_copy(tl["qcs_h"][:, t0:t1, :],
                                  tl["qcs"][:, t0:t1, :])

        def prep_v_amax(bufs, tl):
            """DVE: per-channel |v| max partials (channel-major view)."""
            _, _, v_sb = bufs
            amax_vp = work.tile([128, D], f32, tag="amax_vp")
            nc.vector.tensor_reduce(
                out=amax_vp,
                in_=_bc(v_sb, [v_sb.ap[0], [1, D], [D, T]]),
                axis=mybir.AxisListType.X, op=ALU.max,
                apply_absolute_value=True)
            tl["amax_vp"] = amax_vp

        def alloc_padded(dst_key, tag, tl):
            """Allocate a zero-padded transposed operand + its stack buffer;
            the Pool memset of the pad runs early, off the critical path."""
            tl[dst_key] = opnds.tile([128, T, 128], f16, tag=tag, name=tag)
            nc.gpsimd.memset(tl[dst_key][64:128, :, :], 0.0)
            tl[dst_key + "_st"] = work.tile([128, C, 128], f16,
                                            tag=tag + "_st",
                                            name=tag + "_st")

        def transpose_group(src_key, dst_key, tag, queue, tl, c0=0, c1=None):
            """PE chunk transposes (parity-stacked via DVE) of chunks [c0,c1)
            then two strided parity-split DMAs into the top half of the
            zero-padded [128,(T,128)] operand."""
            c1 = C if c1 is None else c1
            if dst_key not in tl:
                alloc_padded(dst_key, tag, tl)
            dstT, stk = tl[dst_key], tl[dst_key + "_st"]
            x_h = tl[src_key]
            for c in range(c0, c1):
                tp = ps_t.tile([128, 128], f16, tag="pst")
                nc.tensor.transpose(tp, x_h[:, 2 * c:2 * c + 2, :], ident_h)
                nc.vector.tensor_copy(stk[:, c, :], tp)
            eng = nc.sync if queue == "sync" else nc.gpsimd
            d64 = dstT[0:64]
            nci = c1 - c0
            eng.dma_start(
                out=_bc(d64, [d64.ap[0], [2 * 128, nci], [1, 128]],
                        off=c0 * 256),
                in_=stk[0:64, c0:c1, :])
            eng.dma_start(
                out=_bc(d64, [d64.ap[0], [2 * 128, nci], [1, 128]],
                        off=c0 * 256 + 128),
                in_=stk[64:128, c0:c1, :])

        def prep_v_scale_pre(tl):
            """PE transpose of amax partials + DVE scale math (all small)."""
            vt_ps = ps_t.tile([D, 128], f32, tag="pst")
            nc.tensor.transpose(vt_ps, tl["amax_vp"], ident_f)
            amax_vT = scales.tile([D, 1], f32, tag="amax_vT")
            nc.vector.tensor_reduce(out=amax_vT, in_=vt_ps,
                                    axis=mybir.AxisListType.X, op=ALU.max)
            sf_vT = scales.tile([D, 1], f32, tag="sf_vT")
            nc.vector.tensor_scalar(out=sf_vT, in0=amax_vT,
                                    scalar1=1.0 / F8_AMAX_DIV, scalar2=EPS,
                                    op0=ALU.mult, op1=ALU.max)
            rsf_vT = scales.tile([D, 1], f32, tag="rsf_vT")
            nc.vector.reciprocal(rsf_vT, sf_vT)
            # [1,2D] row of (rsf | sf), matmul-broadcast to all partitions
            rs_row = small.tile([1, 2 * D], f32, tag="rs_row")
            nc.sync.dma_start(out=rs_row[:, 0:D], in_=rsf_vT)
            nc.sync.dma_start(out=rs_row[:, D:2 * D], in_=sf_vT)
            rs_bps = ps_t.tile([128, 2 * D], f32, tag="pst")
            nc.tensor.matmul(rs_bps, ones_row, rs_row, start=True, stop=True)
            rs_b = small.tile([128, 2 * D], f32, tag="rs_b")
            nc.vector.tensor_copy(rs_b, rs_bps)
            tl["rs_b"] = rs_b

        def prep_v_quant(bufs, tl, pool=True):
            """fp8 quantize v (scale-multiply + cast to the f8e4 grid).  The
            two big elementwise ops go to Pool in steady state (keeps DVE
            free so the o_ps scale fires promptly)."""
            _, _, v_sb = bufs
            rs_b = tl["rs_b"]
            eng = nc.gpsimd if pool else nc.vector
            vq_pre = work.tile([128, T, D], f32, tag="vq_pre")
            eng.tensor_mul(vq_pre, v_sb,
                           _bc(rs_b, [rs_b.ap[0], [0, T], [1, D]]))
            vq_f8 = work.tile([128, T, D], f8e4, tag="vq_f8")
            eng.tensor_copy(vq_f8, vq_pre)
            tl["vq_f8"] = vq_f8

        def prep_v_aug(tl):
            """DVE: dequantized v (f8 grid * sf_v, rounded to f16) + ones
            column.  With sf_v folded here, the PV output needs no
            per-channel scale (the f16 rounding adds ~5e-4 rel)."""
            rs_b = tl["rs_b"]
            vq_aug = opnds.tile([128, T, D + 1], f16, tag="vq_aug")
            nc.vector.tensor_mul(vq_aug[:, :, 0:D], tl["vq_f8"],
                                 _bc(rs_b, [rs_b.ap[0], [0, T], [1, D]],
                                     off=D))
            nc.gpsimd.memset(vq_aug[:, :, D:D + 1], 1.0)
            tl["vq_aug"] = vq_aug

        def half_loop(h, tl, half, slots=None):
            """Main QK->exp->PV loop for one query half (NQH queries).
            Lookahead: QK(mt+2)/exp(mt+2) emitted before PV(mt).
            slots: {mt: [closure,...]} run after qk_exp(mt+2) is emitted."""
            slots = slots or {}
            kqT, qcsT, vq_aug = tl["kqT"], tl["qcsT"], tl["vq_aug"]
            sf_k = tl["ksf"]
            TH = T // 2

            def qk_exp(mt):
                s_ps = ps_s.tile([128, NQH], f32, tag="pss")
                for j in range(NQH // 512):
                    rhs = qcsT[:, half * TH + 4 * j:half * TH + 4 * (j + 1), :]
                    nc.tensor.matmul(s_ps[:, j * 512:(j + 1) * 512],
                                     kqT[:, mt, :], rhs, start=True, stop=True)
                p_sb = pbuf.tile([128, NQH], f16, tag="p_sb")
                nc.scalar.activation(p_sb, s_ps, ACTF.Exp,
                                     scale=sf_k[:, mt:mt + 1])
                return p_sb

            # Two 4-qtile accumulators (1 psum bank each): PV is emitted
            # query-major (p chunk stationary, vq_aug moving, 65 cols) so the
            # output needs no PE transpose and PV costs 65 cols per qtile.
            QT = NQH // 128
            og = [ps_o.tile([128, QT // 2, 128], f32, tag="pso",
                            name="og")
                  for _ in range(2)]
            ps = [qk_exp(0), qk_exp(1)]
            for mt in range(T):
                if mt + 2 < T:
                    ps.append(qk_exp(mt + 2))
                for fn in slots.get(mt, ()):
                    fn()
                p_sb = ps[mt]
                for i in range(QT):
                    # one accumulation group per psum bank (zero region):
                    # start zeroes the whole bank, so only the first qtile
                    # of each bank starts and only the last stops.
                    nc.tensor.matmul(
                        og[i // (QT // 2)][:, i % (QT // 2), 0:D + 1],
                        p_sb[:, i * 128:(i + 1) * 128],
                        vq_aug[:, mt, :],
                        start=(mt == 0 and i % (QT // 2) == 0),
                        stop=(mt == T - 1 and i % (QT // 2) == QT // 2 - 1))
            return og

        def epilogue_half(og, out_sb, half):
            """Per-qtile denominator divide (pure DVE; no PE involvement).
            Returns one closure per o_ps accumulator group (4 qtiles)."""
            QT = NQH // 128

            def mk(a):
                def group():
                    for i in range(QT // 2):
                        qt = a * (QT // 2) + i
                        rec = ostore.tile([128, 1], f32, tag="rec")
                        nc.vector.reciprocal(rec, og[a][:, i, D:D + 1])
                        nc.vector.tensor_mul(
                            out_sb[:, half * (T // 2) + qt, :],
                            og[a][:, i, 0:D],
                            _bc(rec, [rec.ap[0], [0, D]]))
                return group
            return [mk(0), mk(1)]

        # ---- head pipeline ----
        # Emission schedule (engines execute in emission order, per engine):
        #  half0(h): slot0 alloc pads (h+1, Pool) ; slot1 k cast (h+1, DVE);
        #            slot2 mean (h+1, PE+DVE); slots3-10 one epilogue chunk
        #            of (h-1) half1 each; slot4 k quant chain (h+1, DVE);
        #            slot8 q quant chain; slot11 v amax; slot12 store(h-1).
        #  half1(h): slots1,3 kqT transposes (h+1, 4 chunks each + DMA);
        #            slot2 v scale prefix; slot4 v fp8 quant (Pool);
        #            slots5,7 qcsT transposes; slots8-15 one epilogue chunk
        #            of (h) half0 each; slot13 vq_aug build (DVE).
        # Head 0 is prepped serially (k/q chains split into token halves so
        # the first QK issues after roughly half the quant latency); head 1's
        # prep shifts one slot-group later because head 0's DVE is saturated.
        tl = {}
        bufs = load(0)
        prep_cast_k(bufs, tl)
        mean_pe(tl)
        prep_mean_red(tl)
        TH2 = T // 2
        prep_k_chain(bufs, tl, 0, TH2)
        transpose_group("kq_h", "kqT", "kqT", "sync", tl, 0, C // 2)
        prep_q_chain(bufs, tl, 0, TH2)
        transpose_group("qcs_h", "qcsT", "qcsT", "gpsimd", tl, 0, C // 2)
        prep_v_amax(bufs, tl)
        prep_v_scale_pre(tl)
        prep_v_quant(bufs, tl, pool=False)
        prep_v_aug(tl)

        def k_part2(bufs=bufs, tl=tl):
            prep_k_chain(bufs, tl, TH2, T)

        def kT_part2(tl=tl):
            transpose_group("kq_h", "kqT", "kqT", "sync", tl, C // 2, C)

        def q_part2(bufs=bufs, tl=tl):
            prep_q_chain(bufs, tl, TH2, T)

        def qT_part2(tl=tl):
            transpose_group("qcs_h", "qcsT", "qcsT", "gpsimd", tl, C // 2, C)

        h0_slots0 = {0: [k_part2], 2: [kT_part2], 4: [q_part2],
                     6: [qT_part2]}

        prev_groups1 = None    # divide-group closures: prev head half1
        prev_store = None
        for h in range(heads):
            has_next = h + 1 < heads
            late = 2 if h == 0 else 0    # shift next-head prep on head 0
            out_sb = osb.tile([128, T, D], f32, tag="out_sb")
            slots0 = dict(h0_slots0) if h == 0 else {}
            h0_slots0 = {}
            if prev_groups1 is not None:
                for g in prev_groups1:
                    g()
                slots0.setdefault(12, []).append(prev_store)
            tl_n = {}
            if has_next:
                bufs_n = load(h + 1)

                def s_cast(bufs_n=bufs_n, tl_n=tl_n):
                    prep_cast_k(bufs_n, tl_n)

                def s_mean(tl_n=tl_n):
                    mean_pe(tl_n)
                    prep_mean_red(tl_n)

                def s_kchain(bufs_n=bufs_n, tl_n=tl_n):
                    prep_k_chain(bufs_n, tl_n)

                def s_qchain(bufs_n=bufs_n, tl_n=tl_n):
                    prep_q_chain(bufs_n, tl_n)

                def s_vamax(bufs_n=bufs_n, tl_n=tl_n):
                    prep_v_amax(bufs_n, tl_n)

                def s_alloc(tl_n=tl_n):
                    alloc_padded("kqT", "kqT", tl_n)
                    alloc_padded("qcsT", "qcsT", tl_n)

                slots0.setdefault(0, []).append(s_alloc)
                slots0.setdefault(1, []).append(s_cast)
                slots0.setdefault(2 + late, []).append(s_mean)
                slots0.setdefault(4 + late, []).append(s_kchain)
                slots0.setdefault(8 + late, []).append(s_qchain)
                slots0.setdefault(11 + late, []).append(s_vamax)
            og0 = half_loop(h, tl, 0, slots0)

            def mk_store(h=h, out_sb=out_sb):
                def go():
                    nc.sync.dma_start(
                        out=o_d[h].rearrange("(t p) d -> p t d", p=128),
                        in_=out_sb)
                return go
            # both half-0 divide groups run before half-1's first PV (psum
            # accumulator reuse); pure DVE work right after half-0's last PV,
            # and the ACT exp backlog absorbs the short PE wait.
            for g in epilogue_half(og0, out_sb, 0):
                g()
            slots1 = {}
            if has_next:
                def s_kqT_a(tl_n=tl_n):
                    transpose_group("kq_h", "kqT", "kqT", "sync", tl_n,
                                    0, C // 2)

                def s_kqT_b(tl_n=tl_n):
                    transpose_group("kq_h", "kqT", "kqT", "sync", tl_n,
                                    C // 2, C)

                def s_qcsT_a(tl_n=tl_n):
                    transpose_group("qcs_h", "qcsT", "qcsT", "sync", tl_n,
                                    0, C // 2)

                def s_qcsT_b(tl_n=tl_n):
                    transpose_group("qcs_h", "qcsT", "qcsT", "sync", tl_n,
                                    C // 2, C)

                def s_vpre(tl_n=tl_n):
                    prep_v_scale_pre(tl_n)

                def s_vquant(bufs_n=bufs_n, tl_n=tl_n):
                    prep_v_quant(bufs_n, tl_n, pool=True)

                def s_vaug(tl_n=tl_n):
                    prep_v_aug(tl_n)

                slots1.setdefault(1 + late, []).append(s_kqT_a)
                slots1.setdefault(2 + late, []).append(s_vpre)
                slots1.setdefault(3 + late, []).append(s_kqT_b)
                slots1.setdefault(4 + late, []).append(s_vquant)
                slots1.setdefault(5 + late, []).append(s_qcsT_a)
                slots1.setdefault(7 + late, []).append(s_qcsT_b)
                slots1.setdefault(13, []).append(s_vaug)
            og1 = half_loop(h, tl, 1, slots1)
            prev_groups1 = epilogue_half(og1, out_sb, 1)
            prev_store = mk_store()
            if has_next:
                tl = tl_n
        # last head's half-1 divide tail
        for g in prev_groups1:
            g()
        prev_store()
    return nc


_CACHED = {}


def _get_nc():
    if "nc" not in _CACHED:
        from concourse import bacc

        nc = bacc.Bacc("TRN2", target_bir_lowering=False, debug=False)
        build_attention(nc)
        nc.compile()
        _CACHED["nc"] = nc
    return _CACHED["nc"]


def kernel(q: np.ndarray, k: np.ndarray, v: np.ndarray) -> np.ndarray:
    from concourse.bass_utils import run_bass_kernel_spmd

    nc = _get_nc()
    qf = np.ascontiguousarray(np.asarray(q, dtype=np.float32).reshape(B * H, N, D))
    kf = np.ascontiguousarray(np.asarray(k, dtype=np.float32).reshape(B * H, N, D))
    vf = np.ascontiguousarray(np.asarray(v, dtype=np.float32).reshape(B * H, N, D))
    hpc = HEADS_PER_CORE
    in_maps = [
        {"q": qf[c * hpc:(c + 1) * hpc],
         "k": kf[c * hpc:(c + 1) * hpc],
         "v": vf[c * hpc:(c + 1) * hpc]}
        for c in range(NCORES)
    ]
    res = run_bass_kernel_spmd(nc, in_maps, core_ids=list(range(NCORES)))
    out = np.concatenate([np.asarray(r["out"]) for r in res.results], axis=0)
    return out.reshape(B, H, N, D).astype(np.float32)
